# revision 1
# baseline (speedup 1.0000x reference)
"""Trainium2 Bass kernel for nn_Mesh_Renderer: silhouette rasterizer.

Strategy: data-parallel over batch. Core b renders batch b's 64x64 silhouette
from 1280 triangles. Host-side work is layout only: slice per batch, gather
vertices[faces] (pure indexing, no arithmetic), transpose. All math (camera
transform, perspective divide, edge functions, coverage test, reduction) runs
on device.

Device pipeline per core:
  1. camera basis R from eye (look_at, mirrored op-for-op from the reference)
  2. v_cam = (verts - eye) @ R^T via PE matmuls  (verts pre-gathered per
     face-corner: 1280 faces x 4 corners (a,b,c,a) = 5120 columns)
  3. perspective: x_ndc = x / (z*tan + eps)
  4. edge coefficients per face-edge: e(x,y) = A*x + B*y + C
  5. rasterize 10 face-tiles of 128 faces x 4096 pixels: edge planes as
     K=9 bf16 PE matmuls (coefficients Dekker-split hi/mid/lo in bf16; the
     pixel basis [x,y,1] is exactly bf16, so all products are exact and the
     f32 PSUM accumulation gives f32-class e-values at 1 col/cycle), ACT
     Sign from PSUM -> bf16, min3/max3 chains in bf16 (2x DVE; note walrus
     rejects bf16 tensor_tensor on GPSIMD, so these stay on the DVE),
     acc += sign(min3) - sign(max3) in bf16 (exact small ints)
  6. final PE ones-matmul reduces faces; silhouette = count > -2*F, exact
     since invisible faces are rewritten to the never-covering plane set
     e = (-1, +1, +1) at the coefficient level
"""

import sys

if "/opt/trn_rl_repo" not in sys.path:
    sys.path.insert(0, "/opt/trn_rl_repo")

import numpy as np

import concourse.bacc as bacc
import concourse.tile as tile
from concourse import mybir
from concourse.bass_utils import run_bass_kernel_spmd

F32 = mybir.dt.float32
BF16 = mybir.dt.bfloat16
I32 = mybir.dt.int32
OP = mybir.AluOpType
AF = mybir.ActivationFunctionType

B, V, NF, IMG = 8, 642, 1280, 64
NPIX = IMG * IMG          # 4096
NTILE = NF // 128         # 10 face tiles
NCOL = NF * 4             # 5120 gathered corners (a, b, c, a)
EPS = 1e-8
# tan(deg2rad(15)) in float32, matching jnp.tan(jnp.deg2rad(float32(15)))
TAN_T = float(np.tan(np.deg2rad(np.float32(15.0)).astype(np.float32)))


def _normalize3(nc, pool, v, name):
    """v [1,3] f32 -> v / (||v|| + 1e-8), mirroring the reference formula."""
    sq = pool.tile([1, 3], F32, tag=f"{name}_sq")
    nc.vector.tensor_tensor(sq[:], v[:], v[:], OP.mult)
    s = pool.tile([1, 1], F32, tag=f"{name}_s")
    nc.vector.tensor_reduce(s[:], sq[:], mybir.AxisListType.X, OP.add)
    n = pool.tile([1, 1], F32, tag=f"{name}_n")
    nc.scalar.activation(n[:], s[:], AF.Sqrt)
    # Newton refine sqrt: n1 = 0.5*(n + s/n)
    rn = pool.tile([1, 1], F32, tag=f"{name}_rn")
    nc.vector.reciprocal(rn[:], n[:])
    t = pool.tile([1, 1], F32, tag=f"{name}_t")
    nc.vector.tensor_tensor(t[:], s[:], rn[:], OP.mult)
    t2 = pool.tile([1, 1], F32, tag=f"{name}_t2")
    nc.vector.tensor_tensor(t2[:], n[:], t[:], OP.add)
    n1 = pool.tile([1, 1], F32, tag=f"{name}_n1")
    nc.vector.tensor_scalar(n1[:], t2[:], 0.5, None, OP.mult)
    d = pool.tile([1, 1], F32, tag=f"{name}_d")
    nc.vector.tensor_scalar(d[:], n1[:], EPS, None, OP.add)
    r = pool.tile([1, 1], F32, tag=f"{name}_r")
    nc.vector.reciprocal(r[:], d[:])
    # Newton refine recip: r1 = r*(2 - d*r)
    u = pool.tile([1, 1], F32, tag=f"{name}_u")
    nc.vector.tensor_tensor(u[:], d[:], r[:], OP.mult)
    u2 = pool.tile([1, 1], F32, tag=f"{name}_u2")
    nc.vector.tensor_scalar(u2[:], u[:], -1.0, 2.0, OP.mult, OP.add)
    r1 = pool.tile([1, 1], F32, tag=f"{name}_r1")
    nc.vector.tensor_tensor(r1[:], r[:], u2[:], OP.mult)
    out = pool.tile([1, 3], F32, tag=f"{name}_out")
    nc.vector.tensor_scalar(out[:], v[:], r1[:], None, OP.mult)
    return out


def _cross3(nc, pool, a, b, name):
    """cross(a, b) for [1,3] tiles via duplicated [1,6] buffers."""
    a2 = pool.tile([1, 6], F32, tag=f"{name}_a2")
    nc.vector.tensor_copy(a2[:, 0:3], a[:])
    nc.vector.tensor_copy(a2[:, 3:6], a[:])
    b2 = pool.tile([1, 6], F32, tag=f"{name}_b2")
    nc.vector.tensor_copy(b2[:, 0:3], b[:])
    nc.vector.tensor_copy(b2[:, 3:6], b[:])
    m1 = pool.tile([1, 3], F32, tag=f"{name}_m1")
    nc.vector.tensor_tensor(m1[:], a2[:, 1:4], b2[:, 2:5], OP.mult)
    m2 = pool.tile([1, 3], F32, tag=f"{name}_m2")
    nc.vector.tensor_tensor(m2[:], a2[:, 2:5], b2[:, 1:4], OP.mult)
    out = pool.tile([1, 3], F32, tag=f"{name}_out")
    nc.vector.tensor_tensor(out[:], m1[:], m2[:], OP.subtract)
    return out


def build_kernel(ctx, tc):
    nc = tc.nc
    vgt_d = nc.dram_tensor("vgt", [3, NCOL], F32, kind="ExternalInput")
    eye_d = nc.dram_tensor("eye", [3], F32, kind="ExternalInput")
    sil_d = nc.dram_tensor("sil", [NPIX], F32, kind="ExternalOutput")

    cpool = ctx.enter_context(tc.tile_pool(name="cam", bufs=1))
    ppool = ctx.enter_context(tc.tile_pool(name="proj", bufs=1))
    gpool = ctx.enter_context(tc.tile_pool(name="grid", bufs=1))

    # ---- camera basis (partition 0, tiny tiles) ----
    eyeR = cpool.tile([1, 3], F32)
    nc.sync.dma_start(eyeR[:], eye_d.ap())
    eT = cpool.tile([3, 1], F32)
    nc.sync.dma_start(eT[:], eye_d.ap())

    nege = cpool.tile([1, 3], F32)
    nc.vector.tensor_scalar(nege[:], eyeR[:], -1.0, None, OP.mult)
    z_ax = _normalize3(nc, cpool, nege, "nz")

    xr = cpool.tile([1, 3], F32)
    nc.vector.memset(xr[:], 0.0)
    nc.vector.tensor_copy(xr[:, 0:1], z_ax[:, 2:3])
    nc.vector.tensor_scalar(xr[:, 2:3], z_ax[:, 0:1], -1.0, None, OP.mult)
    x_ax = _normalize3(nc, cpool, xr, "nx")

    yr = _cross3(nc, cpool, z_ax, x_ax, "cy")
    y_ax = _normalize3(nc, cpool, yr, "ny")

    # RT[c, d] = R[d, c]; column d of RT = axis row d
    rt = cpool.tile([3, 3], F32)
    for d, axis in enumerate([x_ax, y_ax, z_ax]):
        nc.sync.dma_start(rt[:, d : d + 1], axis[:])

    # ---- projection of 5120 gathered corners ----
    vca = ppool.tile([128, 120], F32)  # [p, (chunk c, coord d)]
    with tc.tile_pool(name="vg", bufs=1) as vgp, \
         tc.tile_pool(name="pvc", bufs=1, space="PSUM") as psvc:
        vgt = vgp.tile([3, NCOL], F32)
        nc.sync.dma_start(vgt[:], vgt_d.ap())
        vme = vgp.tile([3, NCOL], F32)
        nc.vector.tensor_scalar(vme[:], vgt[:], eT[:], None, OP.subtract)
        vcp = psvc.tile([128, 120], F32)
        for c in range(40):
            nc.tensor.matmul(
                vcp[:, 3 * c : 3 * c + 3],
                vme[:, 128 * c : 128 * (c + 1)],
                rt[:],
                start=True,
                stop=True,
            )
        nc.vector.tensor_copy(vca[:], vcp[:])

    vcav = vca[:].rearrange("p (c d) -> p c d", d=3)
    vx, vy, vz = vcav[:, :, 0], vcav[:, :, 1], vcav[:, :, 2]

    dn = ppool.tile([128, 40], F32)
    nc.vector.tensor_scalar(dn[:], vz, TAN_T, EPS, OP.mult, OP.add)
    rc0 = ppool.tile([128, 40], F32)
    nc.vector.reciprocal(rc0[:], dn[:])
    t = ppool.tile([128, 40], F32)
    nc.vector.tensor_tensor(t[:], dn[:], rc0[:], OP.mult)
    t2 = ppool.tile([128, 40], F32)
    nc.vector.tensor_scalar(t2[:], t[:], -1.0, 2.0, OP.mult, OP.add)
    rc = ppool.tile([128, 40], F32)
    nc.vector.tensor_tensor(rc[:], rc0[:], t2[:], OP.mult)

    xn = ppool.tile([128, 40], F32)
    nc.vector.tensor_tensor(xn[:], vx, rc[:], OP.mult)
    yn = ppool.tile([128, 40], F32)
    nc.vector.tensor_tensor(yn[:], vy, rc[:], OP.mult)

    # ---- edge coefficients: e = A*x + B*y + C per (face, edge) ----
    # Visibility is folded into the coefficients: invisible faces get their
    # projected coords zeroed (A=B=C=0) and then per-edge constant planes
    # e0=-1, e1=e2=+1, which cover nothing under the pos/neg test.
    vz4 = vca[:].rearrange("p (ft k d) -> p ft k d", k=4, d=3)
    mz1 = ppool.tile([128, 10], F32)
    nc.vector.tensor_tensor(mz1[:], vz4[:, :, 0, 2], vz4[:, :, 1, 2], OP.min)
    mz = ppool.tile([128, 10], F32)
    nc.vector.tensor_tensor(mz[:], mz1[:], vz4[:, :, 2, 2], OP.min)
    vg = ppool.tile([128, 10], F32)
    nc.vector.tensor_scalar(vg[:], mz[:], 0.0, None, OP.is_gt)

    xn2 = ppool.tile([128, 40], F32)
    nc.vector.tensor_tensor(
        xn2[:].rearrange("p (ft k) -> p ft k", k=4), 
        xn[:].rearrange("p (ft k) -> p ft k", k=4),
        vg[:].unsqueeze(2).broadcast_to([128, 10, 4]), OP.mult)
    yn2 = ppool.tile([128, 40], F32)
    nc.vector.tensor_tensor(
        yn2[:].rearrange("p (ft k) -> p ft k", k=4),
        yn[:].rearrange("p (ft k) -> p ft k", k=4),
        vg[:].unsqueeze(2).broadcast_to([128, 10, 4]), OP.mult)

    xnv = xn2[:].rearrange("p (ft k) -> p ft k", k=4)
    ynv = yn2[:].rearrange("p (ft k) -> p ft k", k=4)
    # CAB[p, (ft, k, c)]: c = 0/1/2 -> A/B/C for edge k of face 128*ft+p
    CAB = ppool.tile([128, 90], F32)
    CABv = CAB[:].rearrange("p (ft k c) -> p ft k c", k=3, c=3)
    nc.vector.tensor_tensor(CABv[:, :, :, 0], ynv[:, :, 0:3], ynv[:, :, 1:4],
                            OP.subtract)
    nc.vector.tensor_tensor(CABv[:, :, :, 1], xnv[:, :, 1:4], xnv[:, :, 0:3],
                            OP.subtract)
    p1 = ppool.tile([128, 30], F32)
    nc.vector.tensor_tensor(p1[:].rearrange("p (ft k) -> p ft k", k=3),
                            xnv[:, :, 0:3], ynv[:, :, 1:4], OP.mult)
    p2 = ppool.tile([128, 30], F32)
    nc.vector.tensor_tensor(p2[:].rearrange("p (ft k) -> p ft k", k=3),
                            ynv[:, :, 0:3], xnv[:, :, 1:4], OP.mult)
    c0 = ppool.tile([128, 30], F32)
    nc.vector.tensor_tensor(c0[:], p1[:], p2[:], OP.subtract)
    # C offset for invisible faces: (1-vg) * (-1, +1, +1)
    pat = ppool.tile([128, 3], F32)
    nc.vector.memset(pat[:, 0:1], -1.0)
    nc.vector.memset(pat[:, 1:3], 1.0)
    ivg = ppool.tile([128, 10], F32)
    nc.vector.tensor_scalar(ivg[:], vg[:], -1.0, 1.0, OP.mult, OP.add)
    off = ppool.tile([128, 30], F32)
    nc.vector.tensor_tensor(off[:].rearrange("p (ft k) -> p ft k", k=3),
                            ivg[:].unsqueeze(2).broadcast_to([128, 10, 3]),
                            pat[:].unsqueeze(1).broadcast_to([128, 10, 3]),
                            OP.mult)
    nc.vector.tensor_tensor(CABv[:, :, :, 2],
                            c0[:].rearrange("p (ft k) -> p ft k", k=3),
                            off[:].rearrange("p (ft k) -> p ft k", k=3), OP.add)

    # 3-way Dekker split of coefficients into bf16 (hi+mid+lo ~ f32-exact;
    # pixel-grid values are exactly bf16, so bf16 x bf16 products are exact
    # and the K=9 matmul accumulates them in f32 PSUM)
    CAB27 = ppool.tile([128, 270], F32)  # col = ft*27 + k*9 + s*3 + c
    c27 = CAB27[:].rearrange("p (ft k s c) -> p ft k s c", k=3, s=3, c=3)
    hib = ppool.tile([128, 90], BF16)
    nc.vector.tensor_copy(hib[:], CAB[:])                 # hi (bf16 rounded)
    nc.vector.tensor_copy(c27[:, :, :, 0],
                          hib[:].rearrange("p (ft k c) -> p ft k c", k=3, c=3))
    r1 = ppool.tile([128, 90], F32)
    nc.vector.tensor_tensor(r1[:], CAB[:], c27[:, :, :, 0].copy(), OP.subtract)
    mib = ppool.tile([128, 90], BF16)
    nc.vector.tensor_copy(mib[:], r1[:])                  # mid
    nc.vector.tensor_copy(c27[:, :, :, 1],
                          mib[:].rearrange("p (ft k c) -> p ft k c", k=3, c=3))
    r2 = ppool.tile([128, 90], F32)
    nc.vector.tensor_tensor(r2[:], r1[:], c27[:, :, :, 1].copy(), OP.subtract)
    lob = ppool.tile([128, 90], BF16)
    nc.vector.tensor_copy(lob[:], r2[:])                  # lo
    nc.vector.tensor_copy(c27[:, :, :, 2],
                          lob[:].rearrange("p (ft k c) -> p ft k c", k=3, c=3))

    # ---- pixel grids and basis G = [x; y; 1] over raster order (i, j) ----
    it32 = gpool.tile([128, IMG], I32)
    nc.gpsimd.iota(it32[:], pattern=[[1, IMG]], base=0, channel_multiplier=0)
    itf = gpool.tile([128, IMG], F32)
    nc.vector.tensor_copy(itf[:], it32[:])
    xg = gpool.tile([128, IMG], F32)  # x_j = j/32 - 63/64 (exact)
    nc.vector.tensor_scalar(xg[:], itf[:], 1.0 / 32.0, -63.0 / 64.0, OP.mult, OP.add)
    yg = gpool.tile([128, IMG], F32)  # y_i = -x_i
    nc.vector.tensor_scalar(yg[:], xg[:], -1.0, None, OP.mult)
    ones_bf = gpool.tile([128, 1], BF16)
    nc.vector.memset(ones_bf[:], 1.0)
    # stage basis rows on partition 0 (compute engines cannot start at
    # partition>0), then one DMA redistributes to [9, NPIX] bf16 (3 copies of
    # x,y,1 to pair with the hi/mid/lo coefficient rows)
    G9 = gpool.tile([9, NPIX], BF16)
    gst = gpool.tile([1, 3 * NPIX], BF16)
    gsv = gst[:].rearrange("p (r i j) -> p r i j", r=3, i=IMG)
    nc.vector.tensor_copy(gsv[:, 0], xg[0:1, :].unsqueeze(1)
                          .broadcast_to([1, IMG, IMG]))
    nc.vector.tensor_copy(gsv[:, 1], yg[0:1, :].unsqueeze(2)
                          .broadcast_to([1, IMG, IMG]))
    nc.vector.memset(gst[:, 2 * NPIX :], 1.0)
    for rep in range(3):
        nc.sync.dma_start(G9[3 * rep : 3 * rep + 3, :], gst[:])

    # identity for PE transposes
    iop = gpool.tile([128, 1], I32)
    nc.gpsimd.iota(iop[:], pattern=[[1, 1]], base=0, channel_multiplier=1)
    iopf = gpool.tile([128, 1], F32)
    nc.vector.tensor_copy(iopf[:], iop[:])
    iof = gpool.tile([128, 128], I32)
    nc.gpsimd.iota(iof[:], pattern=[[1, 128]], base=0, channel_multiplier=0)
    ioff = gpool.tile([128, 128], F32)
    nc.vector.tensor_copy(ioff[:], iof[:])
    idm = gpool.tile([128, 128], F32)
    nc.vector.tensor_scalar(idm[:], ioff[:], iopf[:], None, OP.is_equal)

    # ---- coefficient transposes: TC[:, (ft*3+k)*128 : +128] = [3, 128] lhsT ----
    TCf = gpool.tile([9, NF * 3], F32)
    with tc.tile_pool(name="ptp", bufs=2, space="PSUM") as ptp:
        for ft in range(NTILE):
            for k in range(3):
                tps = ptp.tile([9, 128], F32, tag="tps")
                nc.tensor.matmul(
                    tps[:], CAB27[:, 27 * ft + 9 * k : 27 * ft + 9 * k + 9],
                    idm[:], start=True, stop=True)
                nc.vector.tensor_copy(
                    TCf[:, (ft * 3 + k) * 128 : (ft * 3 + k + 1) * 128], tps[:])
    TC = gpool.tile([9, NF * 3], BF16)
    nc.vector.tensor_copy(TC[:], TCf[:])

    # ---- rasterization ----
    # Per (face-tile, half, edge): PE matmul (coef lhsT [3,128] x G-half
    # [3,2048]) -> e-plane in PSUM, ACT Sign -> bf16 SBUF. Then smin chain on
    # DVE, smax chain on GPSIMD, acc += smin - smax (bf16, exact ints).
    HALF = NPIX // 2
    spool = ctx.enter_context(tc.tile_pool(name="s3", bufs=2))
    mpool = ctx.enter_context(tc.tile_pool(name="mm", bufs=8))
    apool = ctx.enter_context(tc.tile_pool(name="accp", bufs=4))
    accs = [None, None]
    with tc.tile_pool(name="pe3", bufs=2, space="PSUM") as psE:
        for ft in range(NTILE):
            for h in range(2):
                s3 = spool.tile([128, 3 * HALF], BF16, tag="s3")
                for k in range(3):
                    eps = psE.tile([128, HALF], F32, tag="eps")
                    lhsT = TC[:, (ft * 3 + k) * 128 : (ft * 3 + k + 1) * 128]
                    for q in range(HALF // 512):
                        nc.tensor.matmul(
                            eps[:, 512 * q : 512 * (q + 1)], lhsT,
                            G9[:, HALF * h + 512 * q : HALF * h + 512 * (q + 1)],
                            start=True, stop=True)
                    nc.scalar.activation(s3[:, HALF * k : HALF * (k + 1)],
                                         eps[:], AF.Sign)
                s3r = s3[:].rearrange("p (k x) -> p k x", k=3)
                sm1 = mpool.tile([128, HALF], BF16, tag="mm")
                nc.vector.tensor_tensor(sm1[:], s3r[:, 0], s3r[:, 1], OP.min)
                smin = mpool.tile([128, HALF], BF16, tag="mm")
                nc.vector.tensor_tensor(smin[:], sm1[:], s3r[:, 2], OP.min)
                sM1 = mpool.tile([128, HALF], BF16, tag="mm")
                nc.vector.tensor_tensor(sM1[:], s3r[:, 0], s3r[:, 1], OP.max)
                smax = mpool.tile([128, HALF], BF16, tag="mm")
                nc.vector.tensor_tensor(smax[:], sM1[:], s3r[:, 2], OP.max)
                if accs[h] is None:
                    a = apool.tile([128, HALF], BF16, tag=f"a{h}")
                    nc.vector.tensor_tensor(a[:], smin[:], smax[:], OP.subtract)
                    accs[h] = a
                else:
                    d = mpool.tile([128, HALF], BF16, tag="mm")
                    nc.vector.tensor_tensor(d[:], smin[:], smax[:], OP.subtract)
                    a = apool.tile([128, HALF], BF16, tag=f"a{h}")
                    nc.vector.tensor_tensor(a[:], accs[h][:], d[:], OP.add)
                    accs[h] = a

    # ---- reduce over faces, threshold T > -2F, output ----
    pscnt = ctx.enter_context(tc.tile_pool(name="pcnt", bufs=1, space="PSUM"))
    cnt = pscnt.tile([1, NPIX], F32, tag="cnt")
    for h in range(2):
        for q in range(HALF // 512):
            off2 = HALF * h + 512 * q
            nc.tensor.matmul(cnt[:, off2 : off2 + 512], ones_bf[:],
                             accs[h][:, 512 * q : 512 * (q + 1)],
                             start=True, stop=True)
    silb = gpool.tile([1, NPIX], F32)
    nc.vector.tensor_scalar(silb[:], cnt[:], -2.0 * NF, None, OP.is_gt)
    nc.sync.dma_start(sil_d.ap(), silb[:])


_NC = None


def _get_program():
    global _NC
    if _NC is None:
        nc = bacc.Bacc(
            "TRN2",
            target_bir_lowering=False,
            debug=False,
            enable_asserts=False,
            num_devices=B,
        )
        from contextlib import ExitStack

        with tile.TileContext(nc) as tc:
            with ExitStack() as ctx:
                build_kernel(ctx, tc)
        nc.compile()
        _NC = nc
    return _NC


def _host_layout(vertices, faces):
    """Pure indexing: gather per-face-corner vertices, layout [3, 5120] where
    column n = ft*512 + k*128 + p holds corner k of face ft*128+p."""
    faces4 = np.concatenate([faces, faces[:, :1]], axis=1)  # [1280, 4]
    vidx = faces4.reshape(NTILE, 128, 4).transpose(0, 2, 1).reshape(-1)  # [5120]
    out = []
    for b in range(B):
        vg = vertices[b][vidx]  # [5120, 3]
        out.append(np.ascontiguousarray(vg.T.astype(np.float32)))
    return out


def kernel(vertices, viewpoints, faces, img_size):
    vertices = np.asarray(vertices, dtype=np.float32)
    viewpoints = np.asarray(viewpoints, dtype=np.float32)
    faces = np.asarray(faces, dtype=np.int32)
    assert int(img_size) == IMG and vertices.shape == (B, V, 3)

    nc = _get_program()
    vgts = _host_layout(vertices, faces)
    in_maps = [
        {"vgt": vgts[b], "eye": np.ascontiguousarray(viewpoints[b])}
        for b in range(B)
    ]
    res = run_bass_kernel_spmd(nc, in_maps, core_ids=list(range(B)))
    sil = np.stack([res.results[b]["sil"] for b in range(B)])  # [8, 4096]
    return sil.reshape(B, 1, IMG, IMG).astype(np.float32)


if __name__ == "__main__":
    # quick self-exercise with random data
    rng = np.random.default_rng(0)
    verts = rng.standard_normal((B, V, 3), dtype=np.float32) * 0.5
    vps = rng.standard_normal((B, 3), dtype=np.float32)
    fcs = rng.integers(0, V, (NF, 3), dtype=np.int32)
    out = kernel(verts, vps, fcs, IMG)
    print(out.shape, out.sum())



# revision 2
# speedup vs baseline: 1.0014x; 1.0014x over previous
"""Trainium2 Bass kernel for nn_Mesh_Renderer: silhouette rasterizer.

Data-parallel over batch: core b renders batch b's 64x64 silhouette from 1280
triangles. Host does layout only (per-batch slice + vertex gather); all math
runs on device.

Device pipeline (per core):
  1. camera basis R from eye (look_at, op-for-op from the reference)
  2. projection: K=4 matmul computes R@(v) - R@eye in one accumulation
     (row 3 of lhsT = -1, row 3 of rhs = R@eye), then perspective divide
  3. edge coefficients A,B,C per (face, edge), visibility folded in
     (invisible faces get the never-covering plane set e = (-1,+1,+1))
  4. 3-way Dekker split of coefficients to bf16 (hi/mid/lo); pixel grid is
     exactly bf16, so the K=9 bf16 matmul accumulates exact products in f32
     PSUM -> f32-class e-values
  5. rasterize 4 pixel chunks x 10 face tiles; two per-tile recipes balance
     the ACT and DVE engines:
       T_A: ACT signs of all 3 e-planes, DVE min3/max3 of signs (bf16 2x),
            contribution = min3 - max3 in {0,-1,-2} via +/-ones PE matmuls
       T_1: ACT sign of e2 only; DVE folds e0,e1 by value (min/max straight
            from f32 PSUM, bf16 out - rounding is monotone and
            sign-preserving), then u=is_ge(min,0), w=is_le(max,0) at DVE 4x;
            contribution = u + w in {0,1,2}
     both accumulate into a per-chunk PSUM count via PE matmuls (no DVE
     accumulation ops at all)
  6. threshold count > -2*F_A + 0.5 -> {0,1} silhouette
"""

import sys

if "/opt/trn_rl_repo" not in sys.path:
    sys.path.insert(0, "/opt/trn_rl_repo")

import numpy as np

import concourse.bacc as bacc
import concourse.tile as tile
from concourse import mybir
from concourse.bass_utils import run_bass_kernel_spmd

F32 = mybir.dt.float32
BF16 = mybir.dt.bfloat16
I32 = mybir.dt.int32
OP = mybir.AluOpType
AF = mybir.ActivationFunctionType

B, V, NF, IMG = 8, 642, 1280, 64
NPIX = IMG * IMG          # 4096
NTILE = NF // 128         # 10 face tiles
NCOL = NF * 4             # 5120 gathered corners (a, b, c, a)
EPS = 1e-8
TAN_T = float(np.tan(np.deg2rad(np.float32(15.0)).astype(np.float32)))

CHUNK = 1024              # pixel columns per chunk
NCHUNK = NPIX // CHUNK    # 4
# (ft, chunk) tiles using the T_1 (DVE-heavy) recipe; rest are T_A
T1_SET = frozenset((0, c) for c in range(NCHUNK))
def _thresh(ci):
    n_fa = 128 * sum(1 for ft in range(NTILE) if (ft, ci) not in T1_SET)
    return -2.0 * n_fa + 0.5


def _to_bf16(x):
    import jax.numpy as jnp
    return np.asarray(jnp.asarray(np.asarray(x, dtype=np.float32),
                                  dtype=jnp.bfloat16))


def _grid_const():
    """G9 [9, NPIX] bf16: rows (x, y, 1) x 3 over raster order p = i*64+j."""
    j = np.arange(IMG, dtype=np.float64)
    xg = (2.0 * j - (IMG - 1)) / IMG  # exactly representable in bf16
    xx = np.broadcast_to(xg[None, :], (IMG, IMG)).ravel()
    yy = np.broadcast_to(-xg[:, None], (IMG, IMG)).ravel()
    row = np.stack([xx, yy, np.ones(NPIX)], axis=0).astype(np.float32)
    return _to_bf16(np.concatenate([row, row, row], axis=0))


def _normalize3(nc, pool, v, name):
    """v [1,3] f32 -> v / (||v|| + 1e-8), mirroring the reference formula."""
    sq = pool.tile([1, 3], F32, tag=f"{name}_sq")
    nc.vector.tensor_tensor(sq[:], v[:], v[:], OP.mult)
    s = pool.tile([1, 1], F32, tag=f"{name}_s")
    nc.vector.tensor_reduce(s[:], sq[:], mybir.AxisListType.X, OP.add)
    n = pool.tile([1, 1], F32, tag=f"{name}_n")
    nc.scalar.activation(n[:], s[:], AF.Sqrt)
    rn = pool.tile([1, 1], F32, tag=f"{name}_rn")
    nc.vector.reciprocal(rn[:], n[:])
    t = pool.tile([1, 1], F32, tag=f"{name}_t")
    nc.vector.tensor_tensor(t[:], s[:], rn[:], OP.mult)
    t2 = pool.tile([1, 1], F32, tag=f"{name}_t2")
    nc.vector.tensor_tensor(t2[:], n[:], t[:], OP.add)
    n1 = pool.tile([1, 1], F32, tag=f"{name}_n1")
    nc.vector.tensor_scalar(n1[:], t2[:], 0.5, None, OP.mult)
    d = pool.tile([1, 1], F32, tag=f"{name}_d")
    nc.vector.tensor_scalar(d[:], n1[:], EPS, None, OP.add)
    r = pool.tile([1, 1], F32, tag=f"{name}_r")
    nc.vector.reciprocal(r[:], d[:])
    u = pool.tile([1, 1], F32, tag=f"{name}_u")
    nc.vector.tensor_tensor(u[:], d[:], r[:], OP.mult)
    u2 = pool.tile([1, 1], F32, tag=f"{name}_u2")
    nc.vector.tensor_scalar(u2[:], u[:], -1.0, 2.0, OP.mult, OP.add)
    r1 = pool.tile([1, 1], F32, tag=f"{name}_r1")
    nc.vector.tensor_tensor(r1[:], r[:], u2[:], OP.mult)
    out = pool.tile([1, 3], F32, tag=f"{name}_out")
    nc.vector.tensor_scalar(out[:], v[:], r1[:], None, OP.mult)
    return out


def _cross3(nc, pool, a, b, name):
    a2 = pool.tile([1, 6], F32, tag=f"{name}_a2")
    nc.vector.tensor_copy(a2[:, 0:3], a[:])
    nc.vector.tensor_copy(a2[:, 3:6], a[:])
    b2 = pool.tile([1, 6], F32, tag=f"{name}_b2")
    nc.vector.tensor_copy(b2[:, 0:3], b[:])
    nc.vector.tensor_copy(b2[:, 3:6], b[:])
    m1 = pool.tile([1, 3], F32, tag=f"{name}_m1")
    nc.vector.tensor_tensor(m1[:], a2[:, 1:4], b2[:, 2:5], OP.mult)
    m2 = pool.tile([1, 3], F32, tag=f"{name}_m2")
    nc.vector.tensor_tensor(m2[:], a2[:, 2:5], b2[:, 1:4], OP.mult)
    out = pool.tile([1, 3], F32, tag=f"{name}_out")
    nc.vector.tensor_tensor(out[:], m1[:], m2[:], OP.subtract)
    return out


def build_kernel(ctx, tc):
    from contextlib import ExitStack
    nc = tc.nc
    vgt_d = nc.dram_tensor("vgt", [4, NCOL], F32, kind="ExternalInput")
    eye_d = nc.dram_tensor("eye", [3], F32, kind="ExternalInput")
    sil_d = nc.dram_tensor("sil", [NPIX], F32, kind="ExternalOutput")
    g9_d = nc.inline_tensor(_grid_const(), name="g9c")
    idm_d = nc.inline_tensor(_to_bf16(np.eye(128, dtype=np.float32)),
                             name="idmc")

    cpool = ctx.enter_context(tc.tile_pool(name="cam", bufs=1))
    gpool = ctx.enter_context(tc.tile_pool(name="glob", bufs=1))

    # ---- camera basis (partition 0, tiny tiles); eye DMA first so it is
    # not queued behind the large constant transfers ----
    eyeR = cpool.tile([1, 3], F32)
    nc.sync.dma_start(eyeR[:], eye_d.ap())
    vgt4 = gpool.tile([4, NCOL], F32)
    nc.sync.dma_start(vgt4[:], vgt_d.ap())

    # constants
    G9 = gpool.tile([9, NPIX], BF16)
    nc.sync.dma_start(G9[:], g9_d.ap())
    idm = gpool.tile([128, 128], BF16)
    nc.sync.dma_start(idm[:], idm_d.ap())
    onesP = gpool.tile([128, 1], BF16)
    nc.vector.memset(onesP[:], 1.0)
    negP = gpool.tile([128, 1], BF16)
    nc.vector.memset(negP[:], -1.0)

    nege = cpool.tile([1, 3], F32)
    nc.vector.tensor_scalar(nege[:], eyeR[:], -1.0, None, OP.mult)
    z_ax = _normalize3(nc, cpool, nege, "nz")
    xr = cpool.tile([1, 3], F32)
    nc.vector.memset(xr[:], 0.0)
    nc.vector.tensor_copy(xr[:, 0:1], z_ax[:, 2:3])
    nc.vector.tensor_scalar(xr[:, 2:3], z_ax[:, 0:1], -1.0, None, OP.mult)
    x_ax = _normalize3(nc, cpool, xr, "nx")
    yr = _cross3(nc, cpool, z_ax, x_ax, "cy")
    y_ax = _normalize3(nc, cpool, yr, "ny")

    # rt4 [4,3]: rows 0-2: rt[k,d] = R[d,k] (columns = axes); row 3 = R@eye.
    # Assembled on partition 0 as [1,12] then one DMA (engine ops and the
    # final layout must start at partition 0).
    rtst = cpool.tile([1, 12], F32)
    rtsv = rtst[:].rearrange("p (k d) -> p k d", d=3)
    for d, axis in enumerate([x_ax, y_ax, z_ax]):
        nc.vector.tensor_copy(rtsv[:, 0:3, d], axis[:])
        # rt4[3, d] = dot(axis, eye)
        pr = cpool.tile([1, 3], F32, tag=f"re_pr{d}")
        nc.vector.tensor_tensor(pr[:], axis[:], eyeR[:], OP.mult)
        nc.vector.tensor_reduce(rtsv[:, 3, d : d + 1], pr[:],
                                mybir.AxisListType.X, OP.add)
    rt4 = cpool.tile([4, 3], F32)
    nc.sync.dma_start(rt4[:], rtst[:])

    # ---- projection of 5120 gathered corners: R@(v - eye) via K=4 ----
    ppool = ctx.enter_context(tc.tile_pool(name="proj", bufs=1))
    vca = ppool.tile([128, 120], F32)  # [p, (chunk c, coord d)]
    with tc.tile_pool(name="pvc", bufs=1, space="PSUM") as psvc:
        vcp = psvc.tile([128, 120], F32)
        for c in range(40):
            nc.tensor.matmul(
                vcp[:, 3 * c : 3 * c + 3],
                vgt4[:, 128 * c : 128 * (c + 1)],
                rt4[:],
                start=True,
                stop=True,
            )
        nc.vector.tensor_copy(vca[:], vcp[:])

    vcav = vca[:].rearrange("p (c d) -> p c d", d=3)

    # Full-width tiles; the pipeline below runs twice on ft-column slices so
    # the first face tiles' coefficients are ready before the rest.
    dn = ppool.tile([128, 40], F32)
    rc0 = ppool.tile([128, 40], F32)
    t = ppool.tile([128, 40], F32)
    t2 = ppool.tile([128, 40], F32)
    rc = ppool.tile([128, 40], F32)
    xn = ppool.tile([128, 40], F32)
    yn = ppool.tile([128, 40], F32)
    mz1 = ppool.tile([128, 10], F32)
    mz = ppool.tile([128, 10], F32)
    vg = ppool.tile([128, 10], F32)
    xn2 = ppool.tile([128, 40], F32)
    yn2 = ppool.tile([128, 40], F32)
    CAB = ppool.tile([128, 90], F32)
    p1 = ppool.tile([128, 30], F32)
    p2 = ppool.tile([128, 30], F32)
    c0t = ppool.tile([128, 30], F32)
    ivg = ppool.tile([128, 10], F32)
    off = ppool.tile([128, 30], F32)
    CABbf = ppool.tile([128, 270], BF16)
    hib = ppool.tile([128, 90], BF16)
    hif = ppool.tile([128, 90], F32)
    r1 = ppool.tile([128, 90], F32)
    mib = ppool.tile([128, 90], BF16)
    mif = ppool.tile([128, 90], F32)
    r2 = ppool.tile([128, 90], F32)
    lob = ppool.tile([128, 90], BF16)
    pat = ppool.tile([128, 3], F32)
    nc.vector.memset(pat[:, 0:1], -1.0)
    nc.vector.memset(pat[:, 1:3], 1.0)

    def _coef_slice(f0, f1):
        nf = f1 - f0
        c4 = slice(4 * f0, 4 * f1)           # corner-chunk columns
        fts = slice(f0, f1)
        vzs = vcav[:, 4 * f0 : 4 * f1, :].rearrange(
            "p (ft k) d -> p ft k d", k=4)
        nc.vector.tensor_scalar(dn[:, c4], vcav[:, 4 * f0 : 4 * f1, 2],
                                TAN_T, EPS, OP.mult, OP.add)
        nc.vector.reciprocal(rc0[:, c4], dn[:, c4])
        nc.vector.tensor_tensor(t[:, c4], dn[:, c4], rc0[:, c4], OP.mult)
        nc.vector.tensor_scalar(t2[:, c4], t[:, c4], -1.0, 2.0,
                                OP.mult, OP.add)
        nc.vector.tensor_tensor(rc[:, c4], rc0[:, c4], t2[:, c4], OP.mult)
        nc.vector.tensor_tensor(xn[:, c4], vcav[:, 4 * f0 : 4 * f1, 0],
                                rc[:, c4], OP.mult)
        nc.vector.tensor_tensor(yn[:, c4], vcav[:, 4 * f0 : 4 * f1, 1],
                                rc[:, c4], OP.mult)
        nc.vector.tensor_tensor(mz1[:, fts], vzs[:, :, 0, 2],
                                vzs[:, :, 1, 2], OP.min)
        nc.vector.tensor_tensor(mz[:, fts], mz1[:, fts], vzs[:, :, 2, 2],
                                OP.min)
        nc.vector.tensor_scalar(vg[:, fts], mz[:, fts], 0.0, None, OP.is_gt)
        xnv4 = xn[:, c4].rearrange("p (ft k) -> p ft k", k=4)
        ynv4 = yn[:, c4].rearrange("p (ft k) -> p ft k", k=4)
        vgb = vg[:, fts].unsqueeze(2).broadcast_to([128, nf, 4])
        nc.vector.tensor_tensor(
            xn2[:, c4].rearrange("p (ft k) -> p ft k", k=4), xnv4, vgb,
            OP.mult)
        nc.vector.tensor_tensor(
            yn2[:, c4].rearrange("p (ft k) -> p ft k", k=4), ynv4, vgb,
            OP.mult)
        xnv = xn2[:, c4].rearrange("p (ft k) -> p ft k", k=4)
        ynv = yn2[:, c4].rearrange("p (ft k) -> p ft k", k=4)
        c9 = slice(9 * f0, 9 * f1)
        c3 = slice(3 * f0, 3 * f1)
        CABs = CAB[:, c9].rearrange("p (ft k c) -> p ft k c", k=3, c=3)
        nc.vector.tensor_tensor(CABs[:, :, :, 0], ynv[:, :, 0:3],
                                ynv[:, :, 1:4], OP.subtract)
        nc.vector.tensor_tensor(CABs[:, :, :, 1], xnv[:, :, 1:4],
                                xnv[:, :, 0:3], OP.subtract)
        nc.vector.tensor_tensor(p1[:, c3].rearrange("p (ft k) -> p ft k", k=3),
                                xnv[:, :, 0:3], ynv[:, :, 1:4], OP.mult)
        nc.vector.tensor_tensor(p2[:, c3].rearrange("p (ft k) -> p ft k", k=3),
                                ynv[:, :, 0:3], xnv[:, :, 1:4], OP.mult)
        nc.vector.tensor_tensor(c0t[:, c3], p1[:, c3], p2[:, c3], OP.subtract)
        nc.vector.tensor_scalar(ivg[:, fts], vg[:, fts], -1.0, 1.0,
                                OP.mult, OP.add)
        nc.vector.tensor_tensor(
            off[:, c3].rearrange("p (ft k) -> p ft k", k=3),
            ivg[:, fts].unsqueeze(2).broadcast_to([128, nf, 3]),
            pat[:].unsqueeze(1).broadcast_to([128, nf, 3]), OP.mult)
        nc.vector.tensor_tensor(CABs[:, :, :, 2],
                                c0t[:, c3].rearrange("p (ft k) -> p ft k", k=3),
                                off[:, c3].rearrange("p (ft k) -> p ft k", k=3),
                                OP.add)
        # Dekker split
        cbs = CABbf[:, 27 * f0 : 27 * f1].rearrange(
            "p (ft k s c) -> p ft k s c", k=3, s=3, c=3)
        nc.vector.tensor_copy(hib[:, c9], CAB[:, c9])
        nc.vector.tensor_copy(
            cbs[:, :, :, 0],
            hib[:, c9].rearrange("p (ft k c) -> p ft k c", k=3, c=3))
        nc.vector.tensor_copy(hif[:, c9], hib[:, c9])
        nc.vector.tensor_tensor(r1[:, c9], CAB[:, c9], hif[:, c9], OP.subtract)
        nc.vector.tensor_copy(mib[:, c9], r1[:, c9])
        nc.vector.tensor_copy(
            cbs[:, :, :, 1],
            mib[:, c9].rearrange("p (ft k c) -> p ft k c", k=3, c=3))
        nc.vector.tensor_copy(mif[:, c9], mib[:, c9])
        nc.vector.tensor_tensor(r2[:, c9], r1[:, c9], mif[:, c9], OP.subtract)
        nc.vector.tensor_copy(lob[:, c9], r2[:, c9])
        nc.vector.tensor_copy(
            cbs[:, :, :, 2],
            lob[:, c9].rearrange("p (ft k c) -> p ft k c", k=3, c=3))

    # ---- coefficient transposes: TC_k [9, NF] bf16, lhsT per (ft, k) ----
    TCs = [gpool.tile([9, NF], BF16, name=f"tc{k}") for k in range(3)]

    def _transpose_slice(ptp, f0, f1, dve_only):
        for ft in range(f0, f1):
            for k in range(3):
                tp = ptp.tile([9, 128], BF16, tag="tp")
                nc.tensor.transpose(
                    tp[:], CABbf[:, 27 * ft + 9 * k : 27 * ft + 9 * k + 9],
                    idm[:])
                if dve_only or (ft + k) % 2 == 0:
                    nc.vector.tensor_copy(
                        TCs[k][:, 128 * ft : 128 * (ft + 1)], tp[:])
                else:
                    nc.scalar.activation(
                        TCs[k][:, 128 * ft : 128 * (ft + 1)], tp[:], AF.Copy)

    with tc.tile_pool(name="ptp", bufs=4, space="PSUM") as ptp:
        _coef_slice(0, NTILE)
        _transpose_slice(ptp, 0, NTILE, dve_only=False)

    # ---- rasterization ----
    spool = ctx.enter_context(tc.tile_pool(name="sgn", bufs=4))
    mpool = ctx.enter_context(tc.tile_pool(name="mm", bufs=4))
    silb = gpool.tile([1, NPIX], F32)
    psE = ctx.enter_context(tc.tile_pool(name="pe", bufs=3, space="PSUM"))
    psC = ctx.enter_context(tc.tile_pool(name="pc", bufs=1, space="PSUM"))

    tiles = []
    for ci in range(NCHUNK):
        t1_fts = [f for f in range(NTILE) if (f, ci) in T1_SET]
        ta_fts = [f for f in range(NTILE) if (f, ci) not in T1_SET]
        order = ta_fts[:1] + t1_fts + ta_fts[1:]
        for fi, ft in enumerate(order):
            tiles.append((ci, ft, fi == 0, fi == NTILE - 1))

    cnts = {}
    pend_reduce = None   # (ci, cnt, red, first, last)
    pend_thresh = None   # (ci, cnt)

    def _emit_reduce(pr):
        ci, cnt, red, first, last = pr
        for q in (0, 1):
            for pi, (plane, lhs) in enumerate(red):
                nc.tensor.matmul(
                    cnt[:, 512 * q : 512 * (q + 1)],
                    lhs[:, 0:1],
                    plane[:, 512 * q : 512 * (q + 1)],
                    start=(first and pi == 0),
                    stop=(last and pi == 1))

    def _emit_thresh(ci, cnt):
        c0p = CHUNK * ci
        th = _thresh(ci)
        for q in range(CHUNK // 512):
            nc.vector.tensor_scalar(
                silb[:, c0p + 512 * q : c0p + 512 * (q + 1)],
                cnt[:, 512 * q : 512 * (q + 1)], th, None, OP.is_gt)
        nc.sync.dma_start(sil_d.ap()[c0p : c0p + CHUNK],
                          silb[:, c0p : c0p + CHUNK])

    for ci, ft, first, last in tiles:
        c0p = CHUNK * ci
        if first:
            cnts[ci] = psC.tile([1, CHUNK], F32, name="cnt", tag="cnt")
        cnt = cnts[ci]
        is_t1 = (ft, ci) in T1_SET
        korder = (2, 0, 1) if is_t1 else (0, 1, 2)
        eps = [None, None, None]
        for k in korder:
            ep = psE.tile([128, CHUNK], F32, tag="ep")
            for q in range(CHUNK // 512):
                nc.tensor.matmul(
                    ep[:, 512 * q : 512 * (q + 1)],
                    TCs[k][:, 128 * ft : 128 * (ft + 1)],
                    G9[:, c0p + 512 * q : c0p + 512 * (q + 1)],
                    start=True, stop=True)
            eps[k] = ep
        # software pipeline: the previous tile's reduce-matmuls are emitted
        # only now, so they do not block this tile's e-matmuls in the PE
        # in-order queue; same for the previous chunk's threshold
        if pend_reduce is not None:
            _emit_reduce(pend_reduce)
            pend_reduce = None
        if pend_thresh is not None:
            _emit_thresh(*pend_thresh)
            pend_thresh = None
        if is_t1:
            # T_1: value-domain fold of e0,e1; sign only for e2.
            s2 = spool.tile([128, CHUNK], BF16, tag="s2")
            nc.scalar.activation(s2[:], eps[2][:], AF.Sign)
            cp0 = mpool.tile([128, CHUNK], BF16, tag="cp0")
            nc.vector.tensor_copy(cp0[:], eps[0][:])
            mn1 = mpool.tile([128, CHUNK], BF16, tag="mn1")
            nc.vector.tensor_tensor(mn1[:], cp0[:], eps[1][:], OP.min)
            mx1 = mpool.tile([128, CHUNK], BF16, tag="mx1")
            nc.vector.tensor_tensor(mx1[:], cp0[:], eps[1][:], OP.max)
            m3 = mpool.tile([128, CHUNK], BF16, tag="m3")
            nc.vector.tensor_tensor(m3[:], mn1[:], s2[:], OP.min)
            M3 = mpool.tile([128, CHUNK], BF16, tag="M3")
            nc.vector.tensor_tensor(M3[:], mx1[:], s2[:], OP.max)
            u = mpool.tile([128, CHUNK], BF16, tag="u")
            nc.vector.tensor_scalar(u[:], m3[:], 0.0, None, OP.is_ge)
            w = mpool.tile([128, CHUNK], BF16, tag="w")
            nc.vector.tensor_scalar(w[:], M3[:], 0.0, None, OP.is_le)
            red = [(u, onesP), (w, onesP)]
        else:
            # T_A: signs of all 3, min/max chains in sign domain
            sg = []
            for k in range(3):
                s = spool.tile([128, CHUNK], BF16, tag=f"s{k}")
                nc.scalar.activation(s[:], eps[k][:], AF.Sign)
                sg.append(s)
            m1 = mpool.tile([128, CHUNK], BF16, tag="m1")
            nc.vector.tensor_tensor(m1[:], sg[0][:], sg[1][:], OP.min)
            m3 = mpool.tile([128, CHUNK], BF16, tag="m3")
            nc.vector.tensor_tensor(m3[:], m1[:], sg[2][:], OP.min)
            M1 = mpool.tile([128, CHUNK], BF16, tag="M1")
            nc.vector.tensor_tensor(M1[:], sg[0][:], sg[1][:], OP.max)
            M3 = mpool.tile([128, CHUNK], BF16, tag="M3")
            nc.vector.tensor_tensor(M3[:], M1[:], sg[2][:], OP.max)
            red = [(m3, onesP), (M3, negP)]
        pend_reduce = (ci, cnt, red, first, last)
        if last:
            _emit_reduce(pend_reduce)
            pend_reduce = None
            pend_thresh = (ci, cnt)
    if pend_thresh is not None:
        _emit_thresh(*pend_thresh)


_NC = None


def _get_program():
    global _NC
    if _NC is None:
        nc = bacc.Bacc(
            "TRN2",
            target_bir_lowering=False,
            debug=False,
            enable_asserts=False,
            num_devices=B,
        )
        from contextlib import ExitStack

        with tile.TileContext(nc) as tc:
            with ExitStack() as ctx:
                build_kernel(ctx, tc)
        nc.compile()
        _NC = nc
    return _NC


def _host_layout(vertices, faces):
    """Pure indexing: gather per-face-corner vertices, layout [3, 5120] where
    column n = ft*512 + k*128 + p holds corner k of face ft*128+p."""
    faces4 = np.concatenate([faces, faces[:, :1]], axis=1)  # [1280, 4]
    vidx = faces4.reshape(NTILE, 128, 4).transpose(0, 2, 1).reshape(-1)
    out = []
    for b in range(B):
        vg = vertices[b][vidx]  # [5120, 3]
        v4 = np.concatenate(
            [vg.T.astype(np.float32),
             np.full((1, len(vidx)), -1.0, np.float32)], axis=0)
        out.append(np.ascontiguousarray(v4))
    return out


def kernel(vertices, viewpoints, faces, img_size):
    vertices = np.asarray(vertices, dtype=np.float32)
    viewpoints = np.asarray(viewpoints, dtype=np.float32)
    faces = np.asarray(faces, dtype=np.int32)
    assert int(img_size) == IMG and vertices.shape == (B, V, 3)

    nc = _get_program()
    vgts = _host_layout(vertices, faces)
    in_maps = [
        {"vgt": vgts[b], "eye": np.ascontiguousarray(viewpoints[b])}
        for b in range(B)
    ]
    res = run_bass_kernel_spmd(nc, in_maps, core_ids=list(range(B)))
    sil = np.stack([res.results[b]["sil"] for b in range(B)])  # [8, 4096]
    return sil.reshape(B, 1, IMG, IMG).astype(np.float32)


if __name__ == "__main__":
    rng = np.random.default_rng(0)
    verts = rng.standard_normal((B, V, 3), dtype=np.float32) * 0.5
    vps = rng.standard_normal((B, 3), dtype=np.float32)
    fcs = rng.integers(0, V, (NF, 3), dtype=np.int32)
    out = kernel(verts, vps, fcs, IMG)
    print(out.shape, out.sum())


# revision 3
# speedup vs baseline: 1.0067x; 1.0053x over previous
"""Trainium2 Bass kernel for nn_Mesh_Renderer: silhouette rasterizer.

Data-parallel over batch: core b renders batch b's 64x64 silhouette from 1280
triangles. Host does layout only (per-batch slice + vertex gather); all math
runs on device.

Device pipeline (per core):
  1. camera basis R from eye (look_at, op-for-op from the reference)
  2. projection: K=4 matmul computes R@(v) - R@eye in one accumulation
     (row 3 of lhsT = -1, row 3 of rhs = R@eye), then perspective divide
  3. edge coefficients A,B,C per (face, edge), visibility folded in
     (invisible faces get the never-covering plane set e = (-1,+1,+1))
  4. 3-way Dekker split of coefficients to bf16 (hi/mid/lo); pixel grid is
     exactly bf16, so the K=9 bf16 matmul accumulates exact products in f32
     PSUM -> f32-class e-values
  5. rasterize 4 pixel chunks x 10 face tiles; two per-tile recipes balance
     the ACT and DVE engines:
       T_A: ACT signs of all 3 e-planes, DVE min3/max3 of signs (bf16 2x),
            contribution = min3 - max3 in {0,-1,-2} via +/-ones PE matmuls
       T_1: ACT sign of e2 only; DVE folds e0,e1 by value (min/max straight
            from f32 PSUM, bf16 out - rounding is monotone and
            sign-preserving), then u=is_ge(min,0), w=is_le(max,0) at DVE 4x;
            contribution = u + w in {0,1,2}
     both accumulate into a per-chunk PSUM count via PE matmuls (no DVE
     accumulation ops at all)
  6. threshold count > -2*F_A + 0.5 -> {0,1} silhouette
"""

import sys

if "/opt/trn_rl_repo" not in sys.path:
    sys.path.insert(0, "/opt/trn_rl_repo")

import numpy as np

import concourse.bacc as bacc
import concourse.tile as tile
from concourse import mybir
from concourse.bass_utils import run_bass_kernel_spmd

F32 = mybir.dt.float32
BF16 = mybir.dt.bfloat16
I32 = mybir.dt.int32
OP = mybir.AluOpType
AF = mybir.ActivationFunctionType

B, V, NF, IMG = 8, 642, 1280, 64
NPIX = IMG * IMG          # 4096
NTILE = NF // 128         # 10 face tiles
NCOL = NF * 4             # 5120 gathered corners (a, b, c, a)
EPS = 1e-8
TAN_T = float(np.tan(np.deg2rad(np.float32(15.0)).astype(np.float32)))

CHUNK = 1024              # pixel columns per chunk
NCHUNK = NPIX // CHUNK    # 4
# (ft, chunk) tiles using the T_1 (DVE-heavy) recipe; rest are T_A
T1_SET = frozenset((0, c) for c in range(NCHUNK))
def _thresh(ci):
    n_fa = 128 * sum(1 for ft in range(NTILE) if (ft, ci) not in T1_SET)
    return -2.0 * n_fa + 0.5


def _to_bf16(x):
    import jax.numpy as jnp
    return np.asarray(jnp.asarray(np.asarray(x, dtype=np.float32),
                                  dtype=jnp.bfloat16))


def _grid_const():
    """G9 [9, NPIX] bf16: rows (x, y, 1) x 3 over raster order p = i*64+j."""
    j = np.arange(IMG, dtype=np.float64)
    xg = (2.0 * j - (IMG - 1)) / IMG  # exactly representable in bf16
    xx = np.broadcast_to(xg[None, :], (IMG, IMG)).ravel()
    yy = np.broadcast_to(-xg[:, None], (IMG, IMG)).ravel()
    row = np.stack([xx, yy, np.ones(NPIX)], axis=0).astype(np.float32)
    return _to_bf16(np.concatenate([row, row, row], axis=0))


def _normalize3(nc, pool, v, name):
    """v [1,3] f32 -> v / (||v|| + 1e-8), mirroring the reference formula."""
    sq = pool.tile([1, 3], F32, tag=f"{name}_sq")
    nc.vector.tensor_tensor(sq[:], v[:], v[:], OP.mult)
    s = pool.tile([1, 1], F32, tag=f"{name}_s")
    nc.vector.tensor_reduce(s[:], sq[:], mybir.AxisListType.X, OP.add)
    n = pool.tile([1, 1], F32, tag=f"{name}_n")
    nc.scalar.activation(n[:], s[:], AF.Sqrt)
    rn = pool.tile([1, 1], F32, tag=f"{name}_rn")
    nc.vector.reciprocal(rn[:], n[:])
    t = pool.tile([1, 1], F32, tag=f"{name}_t")
    nc.vector.tensor_tensor(t[:], s[:], rn[:], OP.mult)
    t2 = pool.tile([1, 1], F32, tag=f"{name}_t2")
    nc.vector.tensor_tensor(t2[:], n[:], t[:], OP.add)
    d = pool.tile([1, 1], F32, tag=f"{name}_d")
    nc.vector.tensor_scalar(d[:], t2[:], 0.5, EPS, OP.mult, OP.add)
    r = pool.tile([1, 1], F32, tag=f"{name}_r")
    nc.vector.reciprocal(r[:], d[:])
    u = pool.tile([1, 1], F32, tag=f"{name}_u")
    nc.vector.tensor_tensor(u[:], d[:], r[:], OP.mult)
    u2 = pool.tile([1, 1], F32, tag=f"{name}_u2")
    nc.vector.tensor_scalar(u2[:], u[:], -1.0, 2.0, OP.mult, OP.add)
    r1 = pool.tile([1, 1], F32, tag=f"{name}_r1")
    nc.vector.tensor_tensor(r1[:], r[:], u2[:], OP.mult)
    out = pool.tile([1, 3], F32, tag=f"{name}_out")
    nc.vector.tensor_scalar(out[:], v[:], r1[:], None, OP.mult)
    return out


def _cross3(nc, pool, a, b, name):
    a2 = pool.tile([1, 6], F32, tag=f"{name}_a2")
    nc.vector.tensor_copy(a2[:, 0:3], a[:])
    nc.vector.tensor_copy(a2[:, 3:6], a[:])
    b2 = pool.tile([1, 6], F32, tag=f"{name}_b2")
    nc.vector.tensor_copy(b2[:, 0:3], b[:])
    nc.vector.tensor_copy(b2[:, 3:6], b[:])
    m1 = pool.tile([1, 3], F32, tag=f"{name}_m1")
    nc.vector.tensor_tensor(m1[:], a2[:, 1:4], b2[:, 2:5], OP.mult)
    m2 = pool.tile([1, 3], F32, tag=f"{name}_m2")
    nc.vector.tensor_tensor(m2[:], a2[:, 2:5], b2[:, 1:4], OP.mult)
    out = pool.tile([1, 3], F32, tag=f"{name}_out")
    nc.vector.tensor_tensor(out[:], m1[:], m2[:], OP.subtract)
    return out


def build_kernel(ctx, tc):
    from contextlib import ExitStack
    nc = tc.nc
    vgt_d = nc.dram_tensor("vgt", [4, NCOL], F32, kind="ExternalInput")
    eye_d = nc.dram_tensor("eye", [3], F32, kind="ExternalInput")
    sil_d = nc.dram_tensor("sil", [NPIX], F32, kind="ExternalOutput")
    g9_d = nc.inline_tensor(_grid_const(), name="g9c")
    idm_d = nc.inline_tensor(_to_bf16(np.eye(128, dtype=np.float32)),
                             name="idmc")

    cpool = ctx.enter_context(tc.tile_pool(name="cam", bufs=1))
    gpool = ctx.enter_context(tc.tile_pool(name="glob", bufs=1))

    # ---- camera basis (partition 0, tiny tiles); eye DMA first so it is
    # not queued behind the large constant transfers ----
    eyeR = cpool.tile([1, 3], F32)
    nc.sync.dma_start(eyeR[:], eye_d.ap())
    vgt4 = gpool.tile([4, NCOL], F32)
    nc.sync.dma_start(vgt4[:], vgt_d.ap())

    # constants
    G9 = gpool.tile([9, NPIX], BF16)
    nc.sync.dma_start(G9[:], g9_d.ap())
    idm = gpool.tile([128, 128], BF16)
    nc.sync.dma_start(idm[:], idm_d.ap())
    onesP = gpool.tile([128, 1], BF16)
    nc.vector.memset(onesP[:], 1.0)
    negP = gpool.tile([128, 1], BF16)
    nc.vector.memset(negP[:], -1.0)

    nege = cpool.tile([1, 3], F32)
    nc.vector.tensor_scalar(nege[:], eyeR[:], -1.0, None, OP.mult)
    z_ax = _normalize3(nc, cpool, nege, "nz")
    xr = cpool.tile([1, 3], F32)
    nc.vector.memset(xr[:], 0.0)
    nc.vector.tensor_copy(xr[:, 0:1], z_ax[:, 2:3])
    nc.vector.tensor_scalar(xr[:, 2:3], z_ax[:, 0:1], -1.0, None, OP.mult)
    x_ax = _normalize3(nc, cpool, xr, "nx")
    yr = _cross3(nc, cpool, z_ax, x_ax, "cy")
    y_ax = _normalize3(nc, cpool, yr, "ny")

    # rt4 [4,3]: rows 0-2: rt[k,d] = R[d,k] (columns = axes); row 3 = R@eye.
    # Assembled on partition 0 as [1,12] then one DMA (engine ops and the
    # final layout must start at partition 0).
    rtst = cpool.tile([1, 12], F32)
    rtsv = rtst[:].rearrange("p (k d) -> p k d", d=3)
    for d, axis in enumerate([x_ax, y_ax, z_ax]):
        nc.vector.tensor_copy(rtsv[:, 0:3, d], axis[:])
        # rt4[3, d] = dot(axis, eye)
        pr = cpool.tile([1, 3], F32, tag=f"re_pr{d}")
        nc.vector.tensor_tensor(pr[:], axis[:], eyeR[:], OP.mult)
        nc.vector.tensor_reduce(rtsv[:, 3, d : d + 1], pr[:],
                                mybir.AxisListType.X, OP.add)
    rt4 = cpool.tile([4, 3], F32)
    nc.sync.dma_start(rt4[:], rtst[:])

    # ---- projection of 5120 gathered corners: R@(v - eye) via K=4 ----
    ppool = ctx.enter_context(tc.tile_pool(name="proj", bufs=1))
    vca = ppool.tile([128, 120], F32)  # [p, (chunk c, coord d)]
    with tc.tile_pool(name="pvc", bufs=1, space="PSUM") as psvc:
        vcp = psvc.tile([128, 120], F32)
        for c in range(40):
            nc.tensor.matmul(
                vcp[:, 3 * c : 3 * c + 3],
                vgt4[:, 128 * c : 128 * (c + 1)],
                rt4[:],
                start=True,
                stop=True,
            )
        nc.vector.tensor_copy(vca[:], vcp[:])

    vcav = vca[:].rearrange("p (c d) -> p c d", d=3)

    # Full-width tiles; the pipeline below runs twice on ft-column slices so
    # the first face tiles' coefficients are ready before the rest.
    dn = ppool.tile([128, 40], F32)
    rc0 = ppool.tile([128, 40], F32)
    t = ppool.tile([128, 40], F32)
    t2 = ppool.tile([128, 40], F32)
    rc = ppool.tile([128, 40], F32)
    xn = ppool.tile([128, 40], F32)
    yn = ppool.tile([128, 40], F32)
    mz1 = ppool.tile([128, 10], F32)
    mz = ppool.tile([128, 10], F32)
    vg = ppool.tile([128, 10], F32)
    xn2 = ppool.tile([128, 40], F32)
    yn2 = ppool.tile([128, 40], F32)
    CAB = ppool.tile([128, 90], F32)
    p1 = ppool.tile([128, 30], F32)
    p2 = ppool.tile([128, 30], F32)
    c0t = ppool.tile([128, 30], F32)
    ivg = ppool.tile([128, 10], F32)
    off = ppool.tile([128, 30], F32)
    CABbf = ppool.tile([128, 270], BF16)
    hib = ppool.tile([128, 90], BF16)
    hif = ppool.tile([128, 90], F32)
    r1 = ppool.tile([128, 90], F32)
    mib = ppool.tile([128, 90], BF16)
    mif = ppool.tile([128, 90], F32)
    r2 = ppool.tile([128, 90], F32)
    lob = ppool.tile([128, 90], BF16)
    pat = ppool.tile([128, 3], F32)
    nc.vector.memset(pat[:, 0:1], -1.0)
    nc.vector.memset(pat[:, 1:3], 1.0)

    def _coef_slice(f0, f1):
        nf = f1 - f0
        c4 = slice(4 * f0, 4 * f1)           # corner-chunk columns
        fts = slice(f0, f1)
        vzs = vcav[:, 4 * f0 : 4 * f1, :].rearrange(
            "p (ft k) d -> p ft k d", k=4)
        nc.vector.tensor_scalar(dn[:, c4], vcav[:, 4 * f0 : 4 * f1, 2],
                                TAN_T, EPS, OP.mult, OP.add)
        nc.vector.reciprocal(rc0[:, c4], dn[:, c4])
        nc.vector.tensor_tensor(t[:, c4], dn[:, c4], rc0[:, c4], OP.mult)
        nc.vector.tensor_scalar(t2[:, c4], t[:, c4], -1.0, 2.0,
                                OP.mult, OP.add)
        nc.vector.tensor_tensor(rc[:, c4], rc0[:, c4], t2[:, c4], OP.mult)
        nc.vector.tensor_tensor(xn[:, c4], vcav[:, 4 * f0 : 4 * f1, 0],
                                rc[:, c4], OP.mult)
        nc.vector.tensor_tensor(yn[:, c4], vcav[:, 4 * f0 : 4 * f1, 1],
                                rc[:, c4], OP.mult)
        nc.vector.tensor_tensor(mz1[:, fts], vzs[:, :, 0, 2],
                                vzs[:, :, 1, 2], OP.min)
        nc.vector.tensor_tensor(mz[:, fts], mz1[:, fts], vzs[:, :, 2, 2],
                                OP.min)
        nc.vector.tensor_scalar(vg[:, fts], mz[:, fts], 0.0, None, OP.is_gt)
        xnv4 = xn[:, c4].rearrange("p (ft k) -> p ft k", k=4)
        ynv4 = yn[:, c4].rearrange("p (ft k) -> p ft k", k=4)
        vgb = vg[:, fts].unsqueeze(2).broadcast_to([128, nf, 4])
        nc.vector.tensor_tensor(
            xn2[:, c4].rearrange("p (ft k) -> p ft k", k=4), xnv4, vgb,
            OP.mult)
        nc.vector.tensor_tensor(
            yn2[:, c4].rearrange("p (ft k) -> p ft k", k=4), ynv4, vgb,
            OP.mult)
        xnv = xn2[:, c4].rearrange("p (ft k) -> p ft k", k=4)
        ynv = yn2[:, c4].rearrange("p (ft k) -> p ft k", k=4)
        c9 = slice(9 * f0, 9 * f1)
        c3 = slice(3 * f0, 3 * f1)
        CABs = CAB[:, c9].rearrange("p (ft k c) -> p ft k c", k=3, c=3)
        nc.vector.tensor_tensor(CABs[:, :, :, 0], ynv[:, :, 0:3],
                                ynv[:, :, 1:4], OP.subtract)
        nc.vector.tensor_tensor(CABs[:, :, :, 1], xnv[:, :, 1:4],
                                xnv[:, :, 0:3], OP.subtract)
        nc.vector.tensor_tensor(p1[:, c3].rearrange("p (ft k) -> p ft k", k=3),
                                xnv[:, :, 0:3], ynv[:, :, 1:4], OP.mult)
        nc.vector.tensor_tensor(p2[:, c3].rearrange("p (ft k) -> p ft k", k=3),
                                ynv[:, :, 0:3], xnv[:, :, 1:4], OP.mult)
        nc.vector.tensor_tensor(c0t[:, c3], p1[:, c3], p2[:, c3], OP.subtract)
        nc.vector.tensor_scalar(ivg[:, fts], vg[:, fts], -1.0, 1.0,
                                OP.mult, OP.add)
        nc.vector.tensor_tensor(
            off[:, c3].rearrange("p (ft k) -> p ft k", k=3),
            ivg[:, fts].unsqueeze(2).broadcast_to([128, nf, 3]),
            pat[:].unsqueeze(1).broadcast_to([128, nf, 3]), OP.mult)
        nc.vector.tensor_tensor(CABs[:, :, :, 2],
                                c0t[:, c3].rearrange("p (ft k) -> p ft k", k=3),
                                off[:, c3].rearrange("p (ft k) -> p ft k", k=3),
                                OP.add)
        # Dekker split
        cbs = CABbf[:, 27 * f0 : 27 * f1].rearrange(
            "p (ft k s c) -> p ft k s c", k=3, s=3, c=3)
        nc.vector.tensor_copy(hib[:, c9], CAB[:, c9])
        nc.vector.tensor_copy(
            cbs[:, :, :, 0],
            hib[:, c9].rearrange("p (ft k c) -> p ft k c", k=3, c=3))
        nc.vector.tensor_copy(hif[:, c9], hib[:, c9])
        nc.vector.tensor_tensor(r1[:, c9], CAB[:, c9], hif[:, c9], OP.subtract)
        nc.vector.tensor_copy(mib[:, c9], r1[:, c9])
        nc.vector.tensor_copy(
            cbs[:, :, :, 1],
            mib[:, c9].rearrange("p (ft k c) -> p ft k c", k=3, c=3))
        nc.vector.tensor_copy(mif[:, c9], mib[:, c9])
        nc.vector.tensor_tensor(r2[:, c9], r1[:, c9], mif[:, c9], OP.subtract)
        nc.vector.tensor_copy(lob[:, c9], r2[:, c9])
        nc.vector.tensor_copy(
            cbs[:, :, :, 2],
            lob[:, c9].rearrange("p (ft k c) -> p ft k c", k=3, c=3))

    # ---- coefficient transposes: TC_k [9, NF] bf16, lhsT per (ft, k) ----
    TCs = [gpool.tile([9, NF], BF16, name=f"tc{k}") for k in range(3)]

    def _transpose_slice(ptp, f0, f1, dve_only):
        for ft in range(f0, f1):
            for k in range(3):
                tp = ptp.tile([9, 128], BF16, tag="tp")
                nc.tensor.transpose(
                    tp[:], CABbf[:, 27 * ft + 9 * k : 27 * ft + 9 * k + 9],
                    idm[:])
                if dve_only or (ft + k) % 2 == 0:
                    nc.vector.tensor_copy(
                        TCs[k][:, 128 * ft : 128 * (ft + 1)], tp[:])
                else:
                    nc.scalar.activation(
                        TCs[k][:, 128 * ft : 128 * (ft + 1)], tp[:], AF.Copy)

    with tc.tile_pool(name="ptp", bufs=4, space="PSUM") as ptp:
        _coef_slice(0, NTILE)
        _transpose_slice(ptp, 0, NTILE, dve_only=False)

    # ---- rasterization ----
    spool = ctx.enter_context(tc.tile_pool(name="sgn", bufs=4))
    mpool = ctx.enter_context(tc.tile_pool(name="mm", bufs=4))
    silb = gpool.tile([1, NPIX], F32)
    psE = ctx.enter_context(tc.tile_pool(name="pe", bufs=3, space="PSUM"))
    psC = ctx.enter_context(tc.tile_pool(name="pc", bufs=1, space="PSUM"))

    tiles = []
    for ci in range(NCHUNK):
        t1_fts = [f for f in range(NTILE) if (f, ci) in T1_SET]
        ta_fts = [f for f in range(NTILE) if (f, ci) not in T1_SET]
        order = ta_fts[:1] + t1_fts + ta_fts[1:]
        for fi, ft in enumerate(order):
            tiles.append((ci, ft, fi == 0, fi == NTILE - 1))

    cnts = {}
    pend_reduce = None   # (ci, cnt, red, first, last)
    pend_thresh = None   # (ci, cnt)

    def _emit_reduce(pr):
        ci, cnt, red, first, last = pr
        for q in (0, 1):
            for pi, (plane, lhs) in enumerate(red):
                nc.tensor.matmul(
                    cnt[:, 512 * q : 512 * (q + 1)],
                    lhs[:, 0:1],
                    plane[:, 512 * q : 512 * (q + 1)],
                    start=(first and pi == 0),
                    stop=(last and pi == 1))

    def _emit_thresh(ci, cnt):
        c0p = CHUNK * ci
        th = _thresh(ci)
        for q in range(CHUNK // 512):
            nc.vector.tensor_scalar(
                silb[:, c0p + 512 * q : c0p + 512 * (q + 1)],
                cnt[:, 512 * q : 512 * (q + 1)], th, None, OP.is_gt)
        nc.sync.dma_start(sil_d.ap()[c0p : c0p + CHUNK],
                          silb[:, c0p : c0p + CHUNK])

    for ci, ft, first, last in tiles:
        c0p = CHUNK * ci
        if first:
            cnts[ci] = psC.tile([1, CHUNK], F32, name="cnt", tag="cnt")
        cnt = cnts[ci]
        is_t1 = (ft, ci) in T1_SET
        korder = (2, 0, 1) if is_t1 else (0, 1, 2)
        eps = [None, None, None]
        for k in korder:
            ep = psE.tile([128, CHUNK], F32, tag="ep")
            for q in range(CHUNK // 512):
                nc.tensor.matmul(
                    ep[:, 512 * q : 512 * (q + 1)],
                    TCs[k][:, 128 * ft : 128 * (ft + 1)],
                    G9[:, c0p + 512 * q : c0p + 512 * (q + 1)],
                    start=True, stop=True)
            eps[k] = ep
        # software pipeline: the previous tile's reduce-matmuls are emitted
        # only now, so they do not block this tile's e-matmuls in the PE
        # in-order queue; same for the previous chunk's threshold
        if pend_reduce is not None:
            _emit_reduce(pend_reduce)
            pend_reduce = None
        if pend_thresh is not None:
            _emit_thresh(*pend_thresh)
            pend_thresh = None
        if is_t1:
            # T_1: value-domain fold of e0,e1; sign only for e2.
            s2 = spool.tile([128, CHUNK], BF16, tag="s2")
            nc.scalar.activation(s2[:], eps[2][:], AF.Sign)
            cp0 = mpool.tile([128, CHUNK], BF16, tag="cp0")
            nc.vector.tensor_copy(cp0[:], eps[0][:])
            mn1 = mpool.tile([128, CHUNK], BF16, tag="mn1")
            nc.vector.tensor_tensor(mn1[:], cp0[:], eps[1][:], OP.min)
            mx1 = mpool.tile([128, CHUNK], BF16, tag="mx1")
            nc.vector.tensor_tensor(mx1[:], cp0[:], eps[1][:], OP.max)
            m3 = mpool.tile([128, CHUNK], BF16, tag="m3")
            nc.vector.tensor_tensor(m3[:], mn1[:], s2[:], OP.min)
            M3 = mpool.tile([128, CHUNK], BF16, tag="M3")
            nc.vector.tensor_tensor(M3[:], mx1[:], s2[:], OP.max)
            u = mpool.tile([128, CHUNK], BF16, tag="u")
            nc.vector.tensor_scalar(u[:], m3[:], 0.0, None, OP.is_ge)
            w = mpool.tile([128, CHUNK], BF16, tag="w")
            nc.vector.tensor_scalar(w[:], M3[:], 0.0, None, OP.is_le)
            red = [(u, onesP), (w, onesP)]
        else:
            # T_A: signs of all 3, min/max chains in sign domain
            sg = []
            for k in range(3):
                s = spool.tile([128, CHUNK], BF16, tag=f"s{k}")
                nc.scalar.activation(s[:], eps[k][:], AF.Sign)
                sg.append(s)
            m1 = mpool.tile([128, CHUNK], BF16, tag="m1")
            nc.vector.tensor_tensor(m1[:], sg[0][:], sg[1][:], OP.min)
            m3 = mpool.tile([128, CHUNK], BF16, tag="m3")
            nc.vector.tensor_tensor(m3[:], m1[:], sg[2][:], OP.min)
            M1 = mpool.tile([128, CHUNK], BF16, tag="M1")
            nc.vector.tensor_tensor(M1[:], sg[0][:], sg[1][:], OP.max)
            M3 = mpool.tile([128, CHUNK], BF16, tag="M3")
            nc.vector.tensor_tensor(M3[:], M1[:], sg[2][:], OP.max)
            red = [(m3, onesP), (M3, negP)]
        pend_reduce = (ci, cnt, red, first, last)
        if last:
            _emit_reduce(pend_reduce)
            pend_reduce = None
            pend_thresh = (ci, cnt)
    if pend_thresh is not None:
        _emit_thresh(*pend_thresh)


_NC = None


def _get_program():
    global _NC
    if _NC is None:
        nc = bacc.Bacc(
            "TRN2",
            target_bir_lowering=False,
            debug=False,
            enable_asserts=False,
            num_devices=B,
        )
        from contextlib import ExitStack

        with tile.TileContext(nc) as tc:
            with ExitStack() as ctx:
                build_kernel(ctx, tc)
        nc.compile()
        _NC = nc
    return _NC


def _host_layout(vertices, faces):
    """Pure indexing: gather per-face-corner vertices, layout [3, 5120] where
    column n = ft*512 + k*128 + p holds corner k of face ft*128+p."""
    faces4 = np.concatenate([faces, faces[:, :1]], axis=1)  # [1280, 4]
    vidx = faces4.reshape(NTILE, 128, 4).transpose(0, 2, 1).reshape(-1)
    out = []
    for b in range(B):
        vg = vertices[b][vidx]  # [5120, 3]
        v4 = np.concatenate(
            [vg.T.astype(np.float32),
             np.full((1, len(vidx)), -1.0, np.float32)], axis=0)
        out.append(np.ascontiguousarray(v4))
    return out


def kernel(vertices, viewpoints, faces, img_size):
    vertices = np.asarray(vertices, dtype=np.float32)
    viewpoints = np.asarray(viewpoints, dtype=np.float32)
    faces = np.asarray(faces, dtype=np.int32)
    assert int(img_size) == IMG and vertices.shape == (B, V, 3)

    nc = _get_program()
    vgts = _host_layout(vertices, faces)
    in_maps = [
        {"vgt": vgts[b], "eye": np.ascontiguousarray(viewpoints[b])}
        for b in range(B)
    ]
    res = run_bass_kernel_spmd(nc, in_maps, core_ids=list(range(B)))
    sil = np.stack([res.results[b]["sil"] for b in range(B)])  # [8, 4096]
    return sil.reshape(B, 1, IMG, IMG).astype(np.float32)


if __name__ == "__main__":
    rng = np.random.default_rng(0)
    verts = rng.standard_normal((B, V, 3), dtype=np.float32) * 0.5
    vps = rng.standard_normal((B, 3), dtype=np.float32)
    fcs = rng.integers(0, V, (NF, 3), dtype=np.int32)
    out = kernel(verts, vps, fcs, IMG)
    print(out.shape, out.sum())


# revision 4
# speedup vs baseline: 1.0086x; 1.0019x over previous
"""Trainium2 Bass kernel for nn_Mesh_Renderer: silhouette rasterizer.

Data-parallel over batch: core b renders batch b's 64x64 silhouette from 1280
triangles. Host does layout only (per-batch slice + vertex gather); all math
runs on device.

Device pipeline (per core):
  1. camera basis R from eye (look_at, op-for-op from the reference)
  2. projection: K=4 matmul computes R@(v) - R@eye in one accumulation
     (row 3 of lhsT = -1, row 3 of rhs = R@eye), then perspective divide
  3. edge coefficients A,B,C per (face, edge), visibility folded in
     (invisible faces get the never-covering plane set e = (-1,+1,+1))
  4. 3-way Dekker split of coefficients to bf16 (hi/mid/lo); pixel grid is
     exactly bf16, so the K=9 bf16 matmul accumulates exact products in f32
     PSUM -> f32-class e-values
  5. rasterize 4 pixel chunks x 10 face tiles; two per-tile recipes balance
     the ACT and DVE engines:
       T_A: ACT signs of all 3 e-planes, DVE min3/max3 of signs (bf16 2x),
            contribution = min3 - max3 in {0,-1,-2} via +/-ones PE matmuls
       T_1: ACT sign of e2 only; DVE folds e0,e1 by value (min/max straight
            from f32 PSUM, bf16 out - rounding is monotone and
            sign-preserving), then u=is_ge(min,0), w=is_le(max,0) at DVE 4x;
            contribution = u + w in {0,1,2}
     both accumulate into a per-chunk PSUM count via PE matmuls (no DVE
     accumulation ops at all)
  6. threshold count > -2*F_A + 0.5 -> {0,1} silhouette
"""

import sys

if "/opt/trn_rl_repo" not in sys.path:
    sys.path.insert(0, "/opt/trn_rl_repo")

import numpy as np

import concourse.bacc as bacc
import concourse.tile as tile
from concourse import mybir
from concourse.bass_utils import run_bass_kernel_spmd

F32 = mybir.dt.float32
BF16 = mybir.dt.bfloat16
I32 = mybir.dt.int32
OP = mybir.AluOpType
AF = mybir.ActivationFunctionType

B, V, NF, IMG = 8, 642, 1280, 64
NPIX = IMG * IMG          # 4096
NTILE = NF // 128         # 10 face tiles
NCOL = NF * 4             # 5120 gathered corners (a, b, c, a)
EPS = 1e-8
TAN_T = float(np.tan(np.deg2rad(np.float32(15.0)).astype(np.float32)))

CHUNK = 1024              # pixel columns per chunk
NCHUNK = NPIX // CHUNK    # 4
# (ft, chunk) tiles using the T_1 (DVE-heavy) recipe; rest are T_A
T1_SET = frozenset((0, c) for c in range(NCHUNK))
def _thresh(ci):
    n_fa = 128 * sum(1 for ft in range(NTILE) if (ft, ci) not in T1_SET)
    return -2.0 * n_fa + 0.5


def _to_bf16(x):
    import jax.numpy as jnp
    return np.asarray(jnp.asarray(np.asarray(x, dtype=np.float32),
                                  dtype=jnp.bfloat16))


def _grid_const():
    """G9 [9, NPIX] bf16: rows (x, y, 1) x 3 over raster order p = i*64+j."""
    j = np.arange(IMG, dtype=np.float64)
    xg = (2.0 * j - (IMG - 1)) / IMG  # exactly representable in bf16
    xx = np.broadcast_to(xg[None, :], (IMG, IMG)).ravel()
    yy = np.broadcast_to(-xg[:, None], (IMG, IMG)).ravel()
    row = np.stack([xx, yy, np.ones(NPIX)], axis=0).astype(np.float32)
    return _to_bf16(np.concatenate([row, row, row], axis=0))


def _normalize3(nc, pool, v, name):
    """v [1,3] f32 -> v / (||v|| + 1e-8), mirroring the reference formula."""
    sq = pool.tile([1, 3], F32, tag=f"{name}_sq")
    nc.vector.tensor_tensor(sq[:], v[:], v[:], OP.mult)
    s = pool.tile([1, 1], F32, tag=f"{name}_s")
    nc.vector.tensor_reduce(s[:], sq[:], mybir.AxisListType.X, OP.add)
    n = pool.tile([1, 1], F32, tag=f"{name}_n")
    nc.scalar.activation(n[:], s[:], AF.Sqrt)
    rn = pool.tile([1, 1], F32, tag=f"{name}_rn")
    nc.vector.reciprocal(rn[:], n[:])
    t = pool.tile([1, 1], F32, tag=f"{name}_t")
    nc.vector.tensor_tensor(t[:], s[:], rn[:], OP.mult)
    t2 = pool.tile([1, 1], F32, tag=f"{name}_t2")
    nc.vector.tensor_tensor(t2[:], n[:], t[:], OP.add)
    d = pool.tile([1, 1], F32, tag=f"{name}_d")
    nc.vector.tensor_scalar(d[:], t2[:], 0.5, EPS, OP.mult, OP.add)
    r = pool.tile([1, 1], F32, tag=f"{name}_r")
    nc.vector.reciprocal(r[:], d[:])
    u = pool.tile([1, 1], F32, tag=f"{name}_u")
    nc.vector.tensor_tensor(u[:], d[:], r[:], OP.mult)
    u2 = pool.tile([1, 1], F32, tag=f"{name}_u2")
    nc.vector.tensor_scalar(u2[:], u[:], -1.0, 2.0, OP.mult, OP.add)
    r1 = pool.tile([1, 1], F32, tag=f"{name}_r1")
    nc.vector.tensor_tensor(r1[:], r[:], u2[:], OP.mult)
    out = pool.tile([1, 3], F32, tag=f"{name}_out")
    nc.vector.tensor_scalar(out[:], v[:], r1[:], None, OP.mult)
    return out


def _cross3(nc, pool, a, b, name):
    a2 = pool.tile([1, 6], F32, tag=f"{name}_a2")
    nc.vector.tensor_copy(a2[:, 0:3], a[:])
    nc.vector.tensor_copy(a2[:, 3:6], a[:])
    b2 = pool.tile([1, 6], F32, tag=f"{name}_b2")
    nc.vector.tensor_copy(b2[:, 0:3], b[:])
    nc.vector.tensor_copy(b2[:, 3:6], b[:])
    m1 = pool.tile([1, 3], F32, tag=f"{name}_m1")
    nc.vector.tensor_tensor(m1[:], a2[:, 1:4], b2[:, 2:5], OP.mult)
    m2 = pool.tile([1, 3], F32, tag=f"{name}_m2")
    nc.vector.tensor_tensor(m2[:], a2[:, 2:5], b2[:, 1:4], OP.mult)
    out = pool.tile([1, 3], F32, tag=f"{name}_out")
    nc.vector.tensor_tensor(out[:], m1[:], m2[:], OP.subtract)
    return out


def build_kernel(ctx, tc):
    from contextlib import ExitStack
    nc = tc.nc
    vgt_d = nc.dram_tensor("vgt", [4, NCOL], F32, kind="ExternalInput")
    eye_d = nc.dram_tensor("eye", [3], F32, kind="ExternalInput")
    sil_d = nc.dram_tensor("sil", [NPIX], F32, kind="ExternalOutput")
    g9_d = nc.inline_tensor(_grid_const(), name="g9c")
    idm_d = nc.inline_tensor(_to_bf16(np.eye(128, dtype=np.float32)),
                             name="idmc")

    cpool = ctx.enter_context(tc.tile_pool(name="cam", bufs=1))
    gpool = ctx.enter_context(tc.tile_pool(name="glob", bufs=1))

    # ---- camera basis (partition 0, tiny tiles); eye DMA first so it is
    # not queued behind the large constant transfers ----
    eyeR = cpool.tile([1, 3], F32)
    nc.sync.dma_start(eyeR[:], eye_d.ap())
    vgt4 = gpool.tile([4, NCOL], F32)
    nc.sync.dma_start(vgt4[:], vgt_d.ap())

    # constants
    G9 = gpool.tile([9, NPIX], BF16)
    nc.sync.dma_start(G9[:], g9_d.ap())
    idm = gpool.tile([128, 128], BF16)
    nc.sync.dma_start(idm[:], idm_d.ap())
    onesP = gpool.tile([128, 1], BF16)
    nc.vector.memset(onesP[:], 1.0)
    negP = gpool.tile([128, 1], BF16)
    nc.vector.memset(negP[:], -1.0)

    nege = cpool.tile([1, 3], F32)
    nc.vector.tensor_scalar(nege[:], eyeR[:], -1.0, None, OP.mult)
    z_ax = _normalize3(nc, cpool, nege, "nz")
    xr = cpool.tile([1, 3], F32)
    nc.vector.memset(xr[:], 0.0)
    nc.vector.tensor_copy(xr[:, 0:1], z_ax[:, 2:3])
    nc.vector.tensor_scalar(xr[:, 2:3], z_ax[:, 0:1], -1.0, None, OP.mult)
    x_ax = _normalize3(nc, cpool, xr, "nx")
    y_ax = _cross3(nc, cpool, z_ax, x_ax, "cy")

    # rt4 [4,3]: rows 0-2: rt[k,d] = R[d,k] (columns = axes); row 3 = R@eye.
    # Assembled on partition 0 as [1,12] then one DMA (engine ops and the
    # final layout must start at partition 0).
    rtst = cpool.tile([1, 12], F32)
    rtsv = rtst[:].rearrange("p (k d) -> p k d", d=3)
    for d, axis in enumerate([x_ax, y_ax, z_ax]):
        nc.vector.tensor_copy(rtsv[:, 0:3, d], axis[:])
        # rt4[3, d] = dot(axis, eye)
        pr = cpool.tile([1, 3], F32, tag=f"re_pr{d}")
        nc.vector.tensor_tensor(pr[:], axis[:], eyeR[:], OP.mult)
        nc.vector.tensor_reduce(rtsv[:, 3, d : d + 1], pr[:],
                                mybir.AxisListType.X, OP.add)
    rt4 = cpool.tile([4, 3], F32)
    nc.sync.dma_start(rt4[:], rtst[:])

    # ---- projection of 5120 gathered corners: R@(v - eye) via K=4 ----
    ppool = ctx.enter_context(tc.tile_pool(name="proj", bufs=1))
    vca = ppool.tile([128, 120], F32)  # [p, (chunk c, coord d)]
    with tc.tile_pool(name="pvc", bufs=1, space="PSUM") as psvc:
        vcp = psvc.tile([128, 120], F32)
        for c in range(40):
            nc.tensor.matmul(
                vcp[:, 3 * c : 3 * c + 3],
                vgt4[:, 128 * c : 128 * (c + 1)],
                rt4[:],
                start=True,
                stop=True,
            )
        nc.vector.tensor_copy(vca[:], vcp[:])

    vcav = vca[:].rearrange("p (c d) -> p c d", d=3)

    # Full-width tiles; the pipeline below runs twice on ft-column slices so
    # the first face tiles' coefficients are ready before the rest.
    dn = ppool.tile([128, 40], F32)
    rc0 = ppool.tile([128, 40], F32)
    t = ppool.tile([128, 40], F32)
    t2 = ppool.tile([128, 40], F32)
    rc = ppool.tile([128, 40], F32)
    xn = ppool.tile([128, 40], F32)
    yn = ppool.tile([128, 40], F32)
    mz1 = ppool.tile([128, 10], F32)
    mz = ppool.tile([128, 10], F32)
    vg = ppool.tile([128, 10], F32)
    xn2 = ppool.tile([128, 40], F32)
    yn2 = ppool.tile([128, 40], F32)
    CAB = ppool.tile([128, 90], F32)
    p1 = ppool.tile([128, 30], F32)
    p2 = ppool.tile([128, 30], F32)
    c0t = ppool.tile([128, 30], F32)
    ivg = ppool.tile([128, 10], F32)
    off = ppool.tile([128, 30], F32)
    CABbf = ppool.tile([128, 270], BF16)
    hib = ppool.tile([128, 90], BF16)
    hif = ppool.tile([128, 90], F32)
    r1 = ppool.tile([128, 90], F32)
    mib = ppool.tile([128, 90], BF16)
    mif = ppool.tile([128, 90], F32)
    r2 = ppool.tile([128, 90], F32)
    lob = ppool.tile([128, 90], BF16)
    pat = ppool.tile([128, 3], F32)
    nc.vector.memset(pat[:, 0:1], -1.0)
    nc.vector.memset(pat[:, 1:3], 1.0)

    def _coef_slice(f0, f1):
        nf = f1 - f0
        c4 = slice(4 * f0, 4 * f1)           # corner-chunk columns
        fts = slice(f0, f1)
        vzs = vcav[:, 4 * f0 : 4 * f1, :].rearrange(
            "p (ft k) d -> p ft k d", k=4)
        nc.vector.tensor_scalar(dn[:, c4], vcav[:, 4 * f0 : 4 * f1, 2],
                                TAN_T, EPS, OP.mult, OP.add)
        nc.vector.reciprocal(rc0[:, c4], dn[:, c4])
        nc.vector.tensor_tensor(t[:, c4], dn[:, c4], rc0[:, c4], OP.mult)
        nc.vector.tensor_scalar(t2[:, c4], t[:, c4], -1.0, 2.0,
                                OP.mult, OP.add)
        nc.vector.tensor_tensor(rc[:, c4], rc0[:, c4], t2[:, c4], OP.mult)
        nc.vector.tensor_tensor(xn[:, c4], vcav[:, 4 * f0 : 4 * f1, 0],
                                rc[:, c4], OP.mult)
        nc.vector.tensor_tensor(yn[:, c4], vcav[:, 4 * f0 : 4 * f1, 1],
                                rc[:, c4], OP.mult)
        nc.vector.tensor_tensor(mz1[:, fts], vzs[:, :, 0, 2],
                                vzs[:, :, 1, 2], OP.min)
        nc.vector.tensor_tensor(mz[:, fts], mz1[:, fts], vzs[:, :, 2, 2],
                                OP.min)
        nc.vector.tensor_scalar(vg[:, fts], mz[:, fts], 0.0, None, OP.is_gt)
        xnv4 = xn[:, c4].rearrange("p (ft k) -> p ft k", k=4)
        ynv4 = yn[:, c4].rearrange("p (ft k) -> p ft k", k=4)
        vgb = vg[:, fts].unsqueeze(2).broadcast_to([128, nf, 4])
        nc.vector.tensor_tensor(
            xn2[:, c4].rearrange("p (ft k) -> p ft k", k=4), xnv4, vgb,
            OP.mult)
        nc.vector.tensor_tensor(
            yn2[:, c4].rearrange("p (ft k) -> p ft k", k=4), ynv4, vgb,
            OP.mult)
        xnv = xn2[:, c4].rearrange("p (ft k) -> p ft k", k=4)
        ynv = yn2[:, c4].rearrange("p (ft k) -> p ft k", k=4)
        c9 = slice(9 * f0, 9 * f1)
        c3 = slice(3 * f0, 3 * f1)
        CABs = CAB[:, c9].rearrange("p (ft k c) -> p ft k c", k=3, c=3)
        nc.vector.tensor_tensor(CABs[:, :, :, 0], ynv[:, :, 0:3],
                                ynv[:, :, 1:4], OP.subtract)
        nc.vector.tensor_tensor(CABs[:, :, :, 1], xnv[:, :, 1:4],
                                xnv[:, :, 0:3], OP.subtract)
        nc.vector.tensor_tensor(p1[:, c3].rearrange("p (ft k) -> p ft k", k=3),
                                xnv[:, :, 0:3], ynv[:, :, 1:4], OP.mult)
        nc.vector.tensor_tensor(p2[:, c3].rearrange("p (ft k) -> p ft k", k=3),
                                ynv[:, :, 0:3], xnv[:, :, 1:4], OP.mult)
        nc.vector.tensor_tensor(c0t[:, c3], p1[:, c3], p2[:, c3], OP.subtract)
        nc.vector.tensor_scalar(ivg[:, fts], vg[:, fts], -1.0, 1.0,
                                OP.mult, OP.add)
        nc.vector.tensor_tensor(
            off[:, c3].rearrange("p (ft k) -> p ft k", k=3),
            ivg[:, fts].unsqueeze(2).broadcast_to([128, nf, 3]),
            pat[:].unsqueeze(1).broadcast_to([128, nf, 3]), OP.mult)
        nc.vector.tensor_tensor(CABs[:, :, :, 2],
                                c0t[:, c3].rearrange("p (ft k) -> p ft k", k=3),
                                off[:, c3].rearrange("p (ft k) -> p ft k", k=3),
                                OP.add)
        # Dekker split
        cbs = CABbf[:, 27 * f0 : 27 * f1].rearrange(
            "p (ft k s c) -> p ft k s c", k=3, s=3, c=3)
        nc.vector.tensor_copy(hib[:, c9], CAB[:, c9])
        nc.vector.tensor_copy(
            cbs[:, :, :, 0],
            hib[:, c9].rearrange("p (ft k c) -> p ft k c", k=3, c=3))
        nc.vector.tensor_copy(hif[:, c9], hib[:, c9])
        nc.vector.tensor_tensor(r1[:, c9], CAB[:, c9], hif[:, c9], OP.subtract)
        nc.vector.tensor_copy(mib[:, c9], r1[:, c9])
        nc.vector.tensor_copy(
            cbs[:, :, :, 1],
            mib[:, c9].rearrange("p (ft k c) -> p ft k c", k=3, c=3))
        nc.vector.tensor_copy(mif[:, c9], mib[:, c9])
        nc.vector.tensor_tensor(r2[:, c9], r1[:, c9], mif[:, c9], OP.subtract)
        nc.vector.tensor_copy(lob[:, c9], r2[:, c9])
        nc.vector.tensor_copy(
            cbs[:, :, :, 2],
            lob[:, c9].rearrange("p (ft k c) -> p ft k c", k=3, c=3))

    # ---- coefficient transposes: TC_k [9, NF] bf16, lhsT per (ft, k) ----
    TCs = [gpool.tile([9, NF], BF16, name=f"tc{k}") for k in range(3)]

    def _transpose_slice(ptp, f0, f1, dve_only):
        for ft in range(f0, f1):
            for k in range(3):
                tp = ptp.tile([9, 128], BF16, tag="tp")
                nc.tensor.transpose(
                    tp[:], CABbf[:, 27 * ft + 9 * k : 27 * ft + 9 * k + 9],
                    idm[:])
                if dve_only or (ft + k) % 2 == 0:
                    nc.vector.tensor_copy(
                        TCs[k][:, 128 * ft : 128 * (ft + 1)], tp[:])
                else:
                    nc.scalar.activation(
                        TCs[k][:, 128 * ft : 128 * (ft + 1)], tp[:], AF.Copy)

    with tc.tile_pool(name="ptp", bufs=4, space="PSUM") as ptp:
        _coef_slice(0, NTILE)
        _transpose_slice(ptp, 0, NTILE, dve_only=False)

    # ---- rasterization ----
    spool = ctx.enter_context(tc.tile_pool(name="sgn", bufs=4))
    mpool = ctx.enter_context(tc.tile_pool(name="mm", bufs=4))
    silb = gpool.tile([1, NPIX], F32)
    psE = ctx.enter_context(tc.tile_pool(name="pe", bufs=3, space="PSUM"))
    psC = ctx.enter_context(tc.tile_pool(name="pc", bufs=1, space="PSUM"))

    tiles = []
    for ci in range(NCHUNK):
        t1_fts = [f for f in range(NTILE) if (f, ci) in T1_SET]
        ta_fts = [f for f in range(NTILE) if (f, ci) not in T1_SET]
        order = ta_fts[:1] + t1_fts + ta_fts[1:]
        for fi, ft in enumerate(order):
            tiles.append((ci, ft, fi == 0, fi == NTILE - 1))

    cnts = {}
    pend_reduce = None   # (ci, cnt, red, first, last)
    pend_thresh = None   # (ci, cnt)

    def _emit_reduce(pr):
        ci, cnt, red, first, last = pr
        for q in (0, 1):
            for pi, (plane, lhs) in enumerate(red):
                nc.tensor.matmul(
                    cnt[:, 512 * q : 512 * (q + 1)],
                    lhs[:, 0:1],
                    plane[:, 512 * q : 512 * (q + 1)],
                    start=(first and pi == 0),
                    stop=(last and pi == 1))

    def _emit_thresh(ci, cnt):
        c0p = CHUNK * ci
        th = _thresh(ci)
        for q in range(CHUNK // 512):
            nc.vector.tensor_scalar(
                silb[:, c0p + 512 * q : c0p + 512 * (q + 1)],
                cnt[:, 512 * q : 512 * (q + 1)], th, None, OP.is_gt)
        nc.sync.dma_start(sil_d.ap()[c0p : c0p + CHUNK],
                          silb[:, c0p : c0p + CHUNK])

    for ci, ft, first, last in tiles:
        c0p = CHUNK * ci
        if first:
            cnts[ci] = psC.tile([1, CHUNK], F32, name="cnt", tag="cnt")
        cnt = cnts[ci]
        is_t1 = (ft, ci) in T1_SET
        korder = (2, 0, 1) if is_t1 else (0, 1, 2)
        eps = [None, None, None]
        for k in korder:
            ep = psE.tile([128, CHUNK], F32, tag="ep")
            for q in range(CHUNK // 512):
                nc.tensor.matmul(
                    ep[:, 512 * q : 512 * (q + 1)],
                    TCs[k][:, 128 * ft : 128 * (ft + 1)],
                    G9[:, c0p + 512 * q : c0p + 512 * (q + 1)],
                    start=True, stop=True)
            eps[k] = ep
        # software pipeline: the previous tile's reduce-matmuls are emitted
        # only now, so they do not block this tile's e-matmuls in the PE
        # in-order queue; same for the previous chunk's threshold
        if pend_reduce is not None:
            _emit_reduce(pend_reduce)
            pend_reduce = None
        if pend_thresh is not None:
            _emit_thresh(*pend_thresh)
            pend_thresh = None
        if is_t1:
            # T_1: value-domain fold of e0,e1; sign only for e2.
            s2 = spool.tile([128, CHUNK], BF16, tag="s2")
            nc.scalar.activation(s2[:], eps[2][:], AF.Sign)
            cp0 = mpool.tile([128, CHUNK], BF16, tag="cp0")
            nc.vector.tensor_copy(cp0[:], eps[0][:])
            mn1 = mpool.tile([128, CHUNK], BF16, tag="mn1")
            nc.vector.tensor_tensor(mn1[:], cp0[:], eps[1][:], OP.min)
            mx1 = mpool.tile([128, CHUNK], BF16, tag="mx1")
            nc.vector.tensor_tensor(mx1[:], cp0[:], eps[1][:], OP.max)
            m3 = mpool.tile([128, CHUNK], BF16, tag="m3")
            nc.vector.tensor_tensor(m3[:], mn1[:], s2[:], OP.min)
            M3 = mpool.tile([128, CHUNK], BF16, tag="M3")
            nc.vector.tensor_tensor(M3[:], mx1[:], s2[:], OP.max)
            u = mpool.tile([128, CHUNK], BF16, tag="u")
            nc.vector.tensor_scalar(u[:], m3[:], 0.0, None, OP.is_ge)
            w = mpool.tile([128, CHUNK], BF16, tag="w")
            nc.vector.tensor_scalar(w[:], M3[:], 0.0, None, OP.is_le)
            red = [(u, onesP), (w, onesP)]
        else:
            # T_A: signs of all 3, min/max chains in sign domain
            sg = []
            for k in range(3):
                s = spool.tile([128, CHUNK], BF16, tag=f"s{k}")
                nc.scalar.activation(s[:], eps[k][:], AF.Sign)
                sg.append(s)
            m1 = mpool.tile([128, CHUNK], BF16, tag="m1")
            nc.vector.tensor_tensor(m1[:], sg[0][:], sg[1][:], OP.min)
            m3 = mpool.tile([128, CHUNK], BF16, tag="m3")
            nc.vector.tensor_tensor(m3[:], m1[:], sg[2][:], OP.min)
            M1 = mpool.tile([128, CHUNK], BF16, tag="M1")
            nc.vector.tensor_tensor(M1[:], sg[0][:], sg[1][:], OP.max)
            M3 = mpool.tile([128, CHUNK], BF16, tag="M3")
            nc.vector.tensor_tensor(M3[:], M1[:], sg[2][:], OP.max)
            red = [(m3, onesP), (M3, negP)]
        pend_reduce = (ci, cnt, red, first, last)
        if last:
            _emit_reduce(pend_reduce)
            pend_reduce = None
            pend_thresh = (ci, cnt)
    if pend_thresh is not None:
        _emit_thresh(*pend_thresh)


_NC = None


def _get_program():
    global _NC
    if _NC is None:
        nc = bacc.Bacc(
            "TRN2",
            target_bir_lowering=False,
            debug=False,
            enable_asserts=False,
            num_devices=B,
        )
        from contextlib import ExitStack

        with tile.TileContext(nc) as tc:
            with ExitStack() as ctx:
                build_kernel(ctx, tc)
        nc.compile()
        _NC = nc
    return _NC


def _host_layout(vertices, faces):
    """Pure indexing: gather per-face-corner vertices, layout [3, 5120] where
    column n = ft*512 + k*128 + p holds corner k of face ft*128+p."""
    faces4 = np.concatenate([faces, faces[:, :1]], axis=1)  # [1280, 4]
    vidx = faces4.reshape(NTILE, 128, 4).transpose(0, 2, 1).reshape(-1)
    out = []
    for b in range(B):
        vg = vertices[b][vidx]  # [5120, 3]
        v4 = np.concatenate(
            [vg.T.astype(np.float32),
             np.full((1, len(vidx)), -1.0, np.float32)], axis=0)
        out.append(np.ascontiguousarray(v4))
    return out


def kernel(vertices, viewpoints, faces, img_size):
    vertices = np.asarray(vertices, dtype=np.float32)
    viewpoints = np.asarray(viewpoints, dtype=np.float32)
    faces = np.asarray(faces, dtype=np.int32)
    assert int(img_size) == IMG and vertices.shape == (B, V, 3)

    nc = _get_program()
    vgts = _host_layout(vertices, faces)
    in_maps = [
        {"vgt": vgts[b], "eye": np.ascontiguousarray(viewpoints[b])}
        for b in range(B)
    ]
    res = run_bass_kernel_spmd(nc, in_maps, core_ids=list(range(B)))
    sil = np.stack([res.results[b]["sil"] for b in range(B)])  # [8, 4096]
    return sil.reshape(B, 1, IMG, IMG).astype(np.float32)


if __name__ == "__main__":
    rng = np.random.default_rng(0)
    verts = rng.standard_normal((B, V, 3), dtype=np.float32) * 0.5
    vps = rng.standard_normal((B, 3), dtype=np.float32)
    fcs = rng.integers(0, V, (NF, 3), dtype=np.int32)
    out = kernel(verts, vps, fcs, IMG)
    print(out.shape, out.sum())


# revision 5
# speedup vs baseline: 1.0166x; 1.0079x over previous
"""Trainium2 Bass kernel for nn_Mesh_Renderer: silhouette rasterizer.

Data-parallel over batch: core b renders batch b's 64x64 silhouette from 1280
triangles. Host does layout only (per-batch slice + vertex gather); all math
runs on device.

Device pipeline (per core):
  1. camera basis R from eye (look_at, op-for-op from the reference)
  2. projection: K=4 matmul computes R@(v) - R@eye in one accumulation
     (row 3 of lhsT = -1, row 3 of rhs = R@eye), then perspective divide
  3. edge coefficients A,B,C per (face, edge), visibility folded in
     (invisible faces get the never-covering plane set e = (-1,+1,+1))
  4. 3-way Dekker split of coefficients to bf16 (hi/mid/lo); pixel grid is
     exactly bf16, so the K=9 bf16 matmul accumulates exact products in f32
     PSUM -> f32-class e-values
  5. rasterize 4 pixel chunks x 10 face tiles; two per-tile recipes balance
     the ACT and DVE engines:
       T_A: ACT signs of all 3 e-planes, DVE min3/max3 of signs (bf16 2x),
            contribution = min3 - max3 in {0,-1,-2} via +/-ones PE matmuls
       T_1: ACT sign of e2 only; DVE folds e0,e1 by value (min/max straight
            from f32 PSUM, bf16 out - rounding is monotone and
            sign-preserving), then u=is_ge(min,0), w=is_le(max,0) at DVE 4x;
            contribution = u + w in {0,1,2}
     both accumulate into a per-chunk PSUM count via PE matmuls (no DVE
     accumulation ops at all)
  6. threshold count > -2*F_A + 0.5 -> {0,1} silhouette
"""

import sys

if "/opt/trn_rl_repo" not in sys.path:
    sys.path.insert(0, "/opt/trn_rl_repo")

import numpy as np

import concourse.bacc as bacc
import concourse.tile as tile
from concourse import mybir
from concourse.bass_utils import run_bass_kernel_spmd

F32 = mybir.dt.float32
BF16 = mybir.dt.bfloat16
I32 = mybir.dt.int32
OP = mybir.AluOpType
AF = mybir.ActivationFunctionType

B, V, NF, IMG = 8, 642, 1280, 64
NPIX = IMG * IMG          # 4096
NTILE = NF // 128         # 10 face tiles
NCOL = NF * 4             # 5120 gathered corners (a, b, c, a)
EPS = 1e-8
TAN_T = float(np.tan(np.deg2rad(np.float32(15.0)).astype(np.float32)))

CHUNK = 1024              # pixel columns per chunk
NCHUNK = NPIX // CHUNK    # 4
# (ft, chunk) tiles using the T_1 (DVE-heavy) recipe; rest are T_A
T1_SET = frozenset((0, c) for c in range(NCHUNK))
def _thresh(ci):
    n_fa = 128 * sum(1 for ft in range(NTILE) if (ft, ci) not in T1_SET)
    return -2.0 * n_fa + 0.5


def _to_bf16(x):
    import jax.numpy as jnp
    return np.asarray(jnp.asarray(np.asarray(x, dtype=np.float32),
                                  dtype=jnp.bfloat16))


def _grid_const():
    """G9 [9, NPIX] bf16: rows (x, y, 1) x 3 over raster order p = i*64+j."""
    j = np.arange(IMG, dtype=np.float64)
    xg = (2.0 * j - (IMG - 1)) / IMG  # exactly representable in bf16
    xx = np.broadcast_to(xg[None, :], (IMG, IMG)).ravel()
    yy = np.broadcast_to(-xg[:, None], (IMG, IMG)).ravel()
    row = np.stack([xx, yy, np.ones(NPIX)], axis=0).astype(np.float32)
    return _to_bf16(np.concatenate([row, row, row], axis=0))


def _normalize3(nc, pool, v, name):
    """v [1,3] f32 -> v / (||v|| + 1e-8), mirroring the reference formula.
    Newton refines for sqrt and reciprocal are fused where the rounding
    sequence is provably unchanged (negation is exact)."""
    sq = pool.tile([1, 3], F32, tag=f"{name}_sq")
    nc.vector.tensor_tensor(sq[:], v[:], v[:], OP.mult)
    s = pool.tile([1, 1], F32, tag=f"{name}_s")
    nc.vector.tensor_reduce(s[:], sq[:], mybir.AxisListType.X, OP.add)
    n = pool.tile([1, 1], F32, tag=f"{name}_n")
    nc.scalar.activation(n[:], s[:], AF.Sqrt)
    rn = pool.tile([1, 1], F32, tag=f"{name}_rn")
    nc.vector.reciprocal(rn[:], n[:])
    t2 = pool.tile([1, 1], F32, tag=f"{name}_t2")
    nc.vector.scalar_tensor_tensor(t2[:], s[:], rn[:, 0:1], n[:],
                                   OP.mult, OP.add)
    d = pool.tile([1, 1], F32, tag=f"{name}_d")
    nc.vector.tensor_scalar(d[:], t2[:], 0.5, EPS, OP.mult, OP.add)
    r = pool.tile([1, 1], F32, tag=f"{name}_r")
    nc.vector.reciprocal(r[:], d[:])
    w = pool.tile([1, 1], F32, tag=f"{name}_w")
    nc.vector.scalar_tensor_tensor(w[:], d[:], r[:, 0:1], pool.two3[:, 0:1],
                                   OP.mult, OP.subtract)   # u - 2
    r1n = pool.tile([1, 1], F32, tag=f"{name}_r1n")
    nc.vector.tensor_tensor(r1n[:], r[:], w[:], OP.mult)   # -r1
    out = pool.tile([1, 3], F32, tag=f"{name}_out")
    nc.vector.tensor_scalar(out[:], v[:], r1n[:], -1.0, OP.mult, OP.mult)
    return out


def _cross3(nc, pool, a, b, name):
    a2 = pool.tile([1, 6], F32, tag=f"{name}_a2")
    nc.vector.tensor_copy(a2[:, 0:3], a[:])
    nc.vector.tensor_copy(a2[:, 3:6], a[:])
    b2 = pool.tile([1, 6], F32, tag=f"{name}_b2")
    nc.vector.tensor_copy(b2[:, 0:3], b[:])
    nc.vector.tensor_copy(b2[:, 3:6], b[:])
    m1 = pool.tile([1, 3], F32, tag=f"{name}_m1")
    nc.vector.tensor_tensor(m1[:], a2[:, 1:4], b2[:, 2:5], OP.mult)
    m2 = pool.tile([1, 3], F32, tag=f"{name}_m2")
    nc.vector.tensor_tensor(m2[:], a2[:, 2:5], b2[:, 1:4], OP.mult)
    out = pool.tile([1, 3], F32, tag=f"{name}_out")
    nc.vector.tensor_tensor(out[:], m1[:], m2[:], OP.subtract)
    return out


def build_kernel(ctx, tc):
    from contextlib import ExitStack
    nc = tc.nc
    vgt_d = nc.dram_tensor("vgt", [4, NCOL], F32, kind="ExternalInput")
    eye_d = nc.dram_tensor("eye", [3], F32, kind="ExternalInput")
    sil_d = nc.dram_tensor("sil", [NPIX], F32, kind="ExternalOutput")
    g9_d = nc.inline_tensor(_grid_const(), name="g9c")
    idm_d = nc.inline_tensor(_to_bf16(np.eye(128, dtype=np.float32)),
                             name="idmc")

    cpool = ctx.enter_context(tc.tile_pool(name="cam", bufs=1))
    gpool = ctx.enter_context(tc.tile_pool(name="glob", bufs=1))

    # ---- camera basis (partition 0, tiny tiles); eye DMA first so it is
    # not queued behind the large constant transfers ----
    eyeR = cpool.tile([1, 3], F32)
    nc.sync.dma_start(eyeR[:], eye_d.ap())
    vgt4 = gpool.tile([4, NCOL], F32)
    nc.sync.dma_start(vgt4[:], vgt_d.ap())

    # constants
    G9 = gpool.tile([9, NPIX], BF16)
    nc.sync.dma_start(G9[:], g9_d.ap())
    idm = gpool.tile([128, 128], BF16)
    nc.sync.dma_start(idm[:], idm_d.ap())
    onesP = gpool.tile([128, 1], BF16)
    nc.vector.memset(onesP[:], 1.0)
    negP = gpool.tile([128, 1], BF16)
    nc.vector.memset(negP[:], -1.0)

    two3 = cpool.tile([1, 3], F32)
    nc.vector.memset(two3[:], 2.0)
    cpool.two3 = two3
    nege = cpool.tile([1, 3], F32)
    nc.vector.tensor_scalar(nege[:], eyeR[:], -1.0, None, OP.mult)
    z_ax = _normalize3(nc, cpool, nege, "nz")
    xr = cpool.tile([1, 3], F32)
    nc.vector.memset(xr[:], 0.0)
    nc.vector.tensor_copy(xr[:, 0:1], z_ax[:, 2:3])
    nc.vector.tensor_scalar(xr[:, 2:3], z_ax[:, 0:1], -1.0, None, OP.mult)
    x_ax = _normalize3(nc, cpool, xr, "nx")
    y_ax = _cross3(nc, cpool, z_ax, x_ax, "cy")

    # rt4 [4,3]: rows 0-2: rt[k,d] = R[d,k] (columns = axes); row 3 = R@eye.
    # Assembled on partition 0 as [1,12] then one DMA (engine ops and the
    # final layout must start at partition 0).
    rtst = cpool.tile([1, 12], F32)
    rtsv = rtst[:].rearrange("p (k d) -> p k d", d=3)
    for d, axis in enumerate([x_ax, y_ax, z_ax]):
        nc.vector.tensor_copy(rtsv[:, 0:3, d], axis[:])
        # rt4[3, d] = dot(axis, eye)
        pr = cpool.tile([1, 3], F32, tag=f"re_pr{d}")
        nc.vector.tensor_tensor(pr[:], axis[:], eyeR[:], OP.mult)
        nc.vector.tensor_reduce(rtsv[:, 3, d : d + 1], pr[:],
                                mybir.AxisListType.X, OP.add)
    rt4 = cpool.tile([4, 3], F32)
    nc.sync.dma_start(rt4[:], rtst[:])

    # ---- projection of 5120 gathered corners: R@(v - eye) via K=4 ----
    ppool = ctx.enter_context(tc.tile_pool(name="proj", bufs=1))
    vca = ppool.tile([128, 120], F32)  # [p, (chunk c, coord d)]
    with tc.tile_pool(name="pvc", bufs=1, space="PSUM") as psvc:
        vcp = psvc.tile([128, 120], F32)
        for c in range(40):
            nc.tensor.matmul(
                vcp[:, 3 * c : 3 * c + 3],
                vgt4[:, 128 * c : 128 * (c + 1)],
                rt4[:],
                start=True,
                stop=True,
            )
        nc.vector.tensor_copy(vca[:], vcp[:])

    vcav = vca[:].rearrange("p (c d) -> p c d", d=3)

    # Full-width tiles; the pipeline below runs twice on ft-column slices so
    # the first face tiles' coefficients are ready before the rest.
    dn = ppool.tile([128, 40], F32)
    rc0 = ppool.tile([128, 40], F32)
    t = ppool.tile([128, 40], F32)
    t2 = ppool.tile([128, 40], F32)
    rc = ppool.tile([128, 40], F32)
    xn = ppool.tile([128, 40], F32)
    yn = ppool.tile([128, 40], F32)
    mz1 = ppool.tile([128, 10], F32)
    mz = ppool.tile([128, 10], F32)
    vg = ppool.tile([128, 10], F32)
    xn2 = ppool.tile([128, 40], F32)
    yn2 = ppool.tile([128, 40], F32)
    CAB = ppool.tile([128, 90], F32)
    p1 = ppool.tile([128, 30], F32)
    p2 = ppool.tile([128, 30], F32)
    c0t = ppool.tile([128, 30], F32)
    ivg = ppool.tile([128, 10], F32)
    off = ppool.tile([128, 30], F32)
    CABbf = ppool.tile([128, 270], BF16)
    hib = ppool.tile([128, 90], BF16)
    hif = ppool.tile([128, 90], F32)
    r1 = ppool.tile([128, 90], F32)
    mib = ppool.tile([128, 90], BF16)
    mif = ppool.tile([128, 90], F32)
    r2 = ppool.tile([128, 90], F32)
    lob = ppool.tile([128, 90], BF16)
    pat = ppool.tile([128, 3], F32)
    nc.vector.memset(pat[:, 0:1], -1.0)
    nc.vector.memset(pat[:, 1:3], 1.0)

    def _coef_slice(f0, f1):
        nf = f1 - f0
        c4 = slice(4 * f0, 4 * f1)           # corner-chunk columns
        fts = slice(f0, f1)
        vzs = vcav[:, 4 * f0 : 4 * f1, :].rearrange(
            "p (ft k) d -> p ft k d", k=4)
        nc.vector.tensor_scalar(dn[:, c4], vcav[:, 4 * f0 : 4 * f1, 2],
                                TAN_T, EPS, OP.mult, OP.add)
        nc.vector.reciprocal(rc0[:, c4], dn[:, c4])
        nc.vector.tensor_tensor(t[:, c4], dn[:, c4], rc0[:, c4], OP.mult)
        nc.vector.tensor_scalar(t2[:, c4], t[:, c4], -1.0, 2.0,
                                OP.mult, OP.add)
        nc.vector.tensor_tensor(rc[:, c4], rc0[:, c4], t2[:, c4], OP.mult)
        nc.vector.tensor_tensor(xn[:, c4], vcav[:, 4 * f0 : 4 * f1, 0],
                                rc[:, c4], OP.mult)
        nc.vector.tensor_tensor(yn[:, c4], vcav[:, 4 * f0 : 4 * f1, 1],
                                rc[:, c4], OP.mult)
        nc.vector.tensor_tensor(mz1[:, fts], vzs[:, :, 0, 2],
                                vzs[:, :, 1, 2], OP.min)
        nc.vector.tensor_tensor(mz[:, fts], mz1[:, fts], vzs[:, :, 2, 2],
                                OP.min)
        nc.vector.tensor_scalar(vg[:, fts], mz[:, fts], 0.0, None, OP.is_gt)
        xnv4 = xn[:, c4].rearrange("p (ft k) -> p ft k", k=4)
        ynv4 = yn[:, c4].rearrange("p (ft k) -> p ft k", k=4)
        vgb = vg[:, fts].unsqueeze(2).broadcast_to([128, nf, 4])
        nc.vector.tensor_tensor(
            xn2[:, c4].rearrange("p (ft k) -> p ft k", k=4), xnv4, vgb,
            OP.mult)
        nc.vector.tensor_tensor(
            yn2[:, c4].rearrange("p (ft k) -> p ft k", k=4), ynv4, vgb,
            OP.mult)
        xnv = xn2[:, c4].rearrange("p (ft k) -> p ft k", k=4)
        ynv = yn2[:, c4].rearrange("p (ft k) -> p ft k", k=4)
        c9 = slice(9 * f0, 9 * f1)
        c3 = slice(3 * f0, 3 * f1)
        CABs = CAB[:, c9].rearrange("p (ft k c) -> p ft k c", k=3, c=3)
        nc.vector.tensor_tensor(CABs[:, :, :, 0], ynv[:, :, 0:3],
                                ynv[:, :, 1:4], OP.subtract)
        nc.vector.tensor_tensor(CABs[:, :, :, 1], xnv[:, :, 1:4],
                                xnv[:, :, 0:3], OP.subtract)
        nc.vector.tensor_tensor(p1[:, c3].rearrange("p (ft k) -> p ft k", k=3),
                                xnv[:, :, 0:3], ynv[:, :, 1:4], OP.mult)
        nc.vector.tensor_tensor(p2[:, c3].rearrange("p (ft k) -> p ft k", k=3),
                                ynv[:, :, 0:3], xnv[:, :, 1:4], OP.mult)
        nc.vector.tensor_tensor(c0t[:, c3], p1[:, c3], p2[:, c3], OP.subtract)
        nc.vector.tensor_scalar(ivg[:, fts], vg[:, fts], -1.0, 1.0,
                                OP.mult, OP.add)
        nc.vector.tensor_tensor(
            off[:, c3].rearrange("p (ft k) -> p ft k", k=3),
            ivg[:, fts].unsqueeze(2).broadcast_to([128, nf, 3]),
            pat[:].unsqueeze(1).broadcast_to([128, nf, 3]), OP.mult)
        nc.vector.tensor_tensor(CABs[:, :, :, 2],
                                c0t[:, c3].rearrange("p (ft k) -> p ft k", k=3),
                                off[:, c3].rearrange("p (ft k) -> p ft k", k=3),
                                OP.add)
        # Dekker split
        cbs = CABbf[:, 27 * f0 : 27 * f1].rearrange(
            "p (ft k s c) -> p ft k s c", k=3, s=3, c=3)
        nc.vector.tensor_copy(hib[:, c9], CAB[:, c9])
        nc.vector.tensor_copy(
            cbs[:, :, :, 0],
            hib[:, c9].rearrange("p (ft k c) -> p ft k c", k=3, c=3))
        nc.vector.tensor_copy(hif[:, c9], hib[:, c9])
        nc.vector.tensor_tensor(r1[:, c9], CAB[:, c9], hif[:, c9], OP.subtract)
        nc.vector.tensor_copy(mib[:, c9], r1[:, c9])
        nc.vector.tensor_copy(
            cbs[:, :, :, 1],
            mib[:, c9].rearrange("p (ft k c) -> p ft k c", k=3, c=3))
        nc.vector.tensor_copy(mif[:, c9], mib[:, c9])
        nc.vector.tensor_tensor(r2[:, c9], r1[:, c9], mif[:, c9], OP.subtract)
        nc.vector.tensor_copy(lob[:, c9], r2[:, c9])
        nc.vector.tensor_copy(
            cbs[:, :, :, 2],
            lob[:, c9].rearrange("p (ft k c) -> p ft k c", k=3, c=3))

    # ---- coefficient transposes: TC_k [9, NF] bf16, lhsT per (ft, k) ----
    TCs = [gpool.tile([9, NF], BF16, name=f"tc{k}") for k in range(3)]

    def _transpose_slice(ptp, f0, f1, dve_only):
        for ft in range(f0, f1):
            for k in range(3):
                tp = ptp.tile([9, 128], BF16, tag="tp")
                nc.tensor.transpose(
                    tp[:], CABbf[:, 27 * ft + 9 * k : 27 * ft + 9 * k + 9],
                    idm[:])
                if dve_only or (ft + k) % 2 == 0:
                    nc.vector.tensor_copy(
                        TCs[k][:, 128 * ft : 128 * (ft + 1)], tp[:])
                else:
                    nc.scalar.activation(
                        TCs[k][:, 128 * ft : 128 * (ft + 1)], tp[:], AF.Copy)

    with tc.tile_pool(name="ptp", bufs=4, space="PSUM") as ptp:
        _coef_slice(0, NTILE)
        _transpose_slice(ptp, 0, NTILE, dve_only=False)

    # ---- rasterization ----
    spool = ctx.enter_context(tc.tile_pool(name="sgn", bufs=4))
    mpool = ctx.enter_context(tc.tile_pool(name="mm", bufs=4))
    silb = gpool.tile([1, NPIX], F32)
    psE = ctx.enter_context(tc.tile_pool(name="pe", bufs=3, space="PSUM"))
    psC = ctx.enter_context(tc.tile_pool(name="pc", bufs=1, space="PSUM"))

    tiles = []
    for ci in range(NCHUNK):
        t1_fts = [f for f in range(NTILE) if (f, ci) in T1_SET]
        ta_fts = [f for f in range(NTILE) if (f, ci) not in T1_SET]
        order = ta_fts[:1] + t1_fts + ta_fts[1:]
        for fi, ft in enumerate(order):
            tiles.append((ci, ft, fi == 0, fi == NTILE - 1))

    cnts = {}
    pend_reduce = None   # (ci, cnt, red, first, last)
    pend_thresh = None   # (ci, cnt)

    def _emit_reduce(pr):
        ci, cnt, red, first, last = pr
        for q in (0, 1):
            for pi, (plane, lhs) in enumerate(red):
                nc.tensor.matmul(
                    cnt[:, 512 * q : 512 * (q + 1)],
                    lhs[:, 0:1],
                    plane[:, 512 * q : 512 * (q + 1)],
                    start=(first and pi == 0),
                    stop=(last and pi == 1))

    def _emit_thresh(ci, cnt):
        c0p = CHUNK * ci
        th = _thresh(ci)
        for q in range(CHUNK // 512):
            nc.vector.tensor_scalar(
                silb[:, c0p + 512 * q : c0p + 512 * (q + 1)],
                cnt[:, 512 * q : 512 * (q + 1)], th, None, OP.is_gt)
        nc.sync.dma_start(sil_d.ap()[c0p : c0p + CHUNK],
                          silb[:, c0p : c0p + CHUNK])

    for ci, ft, first, last in tiles:
        c0p = CHUNK * ci
        if first:
            cnts[ci] = psC.tile([1, CHUNK], F32, name="cnt", tag="cnt")
        cnt = cnts[ci]
        is_t1 = (ft, ci) in T1_SET
        korder = (2, 0, 1) if is_t1 else (0, 1, 2)
        eps = [None, None, None]
        for k in korder:
            ep = psE.tile([128, CHUNK], F32, tag="ep")
            for q in range(CHUNK // 512):
                nc.tensor.matmul(
                    ep[:, 512 * q : 512 * (q + 1)],
                    TCs[k][:, 128 * ft : 128 * (ft + 1)],
                    G9[:, c0p + 512 * q : c0p + 512 * (q + 1)],
                    start=True, stop=True)
            eps[k] = ep
        # software pipeline: the previous tile's reduce-matmuls are emitted
        # only now, so they do not block this tile's e-matmuls in the PE
        # in-order queue; same for the previous chunk's threshold
        if pend_reduce is not None:
            _emit_reduce(pend_reduce)
            pend_reduce = None
        if pend_thresh is not None:
            _emit_thresh(*pend_thresh)
            pend_thresh = None
        if is_t1:
            # T_1: value-domain fold of e0,e1; sign only for e2.
            s2 = spool.tile([128, CHUNK], BF16, tag="s2")
            nc.scalar.activation(s2[:], eps[2][:], AF.Sign)
            cp0 = mpool.tile([128, CHUNK], BF16, tag="cp0")
            nc.vector.tensor_copy(cp0[:], eps[0][:])
            mn1 = mpool.tile([128, CHUNK], BF16, tag="mn1")
            nc.vector.tensor_tensor(mn1[:], cp0[:], eps[1][:], OP.min)
            mx1 = mpool.tile([128, CHUNK], BF16, tag="mx1")
            nc.vector.tensor_tensor(mx1[:], cp0[:], eps[1][:], OP.max)
            m3 = mpool.tile([128, CHUNK], BF16, tag="m3")
            nc.vector.tensor_tensor(m3[:], mn1[:], s2[:], OP.min)
            M3 = mpool.tile([128, CHUNK], BF16, tag="M3")
            nc.vector.tensor_tensor(M3[:], mx1[:], s2[:], OP.max)
            u = mpool.tile([128, CHUNK], BF16, tag="u")
            nc.vector.tensor_scalar(u[:], m3[:], 0.0, None, OP.is_ge)
            w = mpool.tile([128, CHUNK], BF16, tag="w")
            nc.vector.tensor_scalar(w[:], M3[:], 0.0, None, OP.is_le)
            red = [(u, onesP), (w, onesP)]
        else:
            # T_A: signs of all 3, min/max chains in sign domain
            sg = []
            for k in range(3):
                s = spool.tile([128, CHUNK], BF16, tag=f"s{k}")
                nc.scalar.activation(s[:], eps[k][:], AF.Sign)
                sg.append(s)
            m1 = mpool.tile([128, CHUNK], BF16, tag="m1")
            nc.vector.tensor_tensor(m1[:], sg[0][:], sg[1][:], OP.min)
            m3 = mpool.tile([128, CHUNK], BF16, tag="m3")
            nc.vector.tensor_tensor(m3[:], m1[:], sg[2][:], OP.min)
            M1 = mpool.tile([128, CHUNK], BF16, tag="M1")
            nc.vector.tensor_tensor(M1[:], sg[0][:], sg[1][:], OP.max)
            M3 = mpool.tile([128, CHUNK], BF16, tag="M3")
            nc.vector.tensor_tensor(M3[:], M1[:], sg[2][:], OP.max)
            red = [(m3, onesP), (M3, negP)]
        pend_reduce = (ci, cnt, red, first, last)
        if last:
            _emit_reduce(pend_reduce)
            pend_reduce = None
            pend_thresh = (ci, cnt)
    if pend_thresh is not None:
        _emit_thresh(*pend_thresh)


_NC = None


def _get_program():
    global _NC
    if _NC is None:
        nc = bacc.Bacc(
            "TRN2",
            target_bir_lowering=False,
            debug=False,
            enable_asserts=False,
            num_devices=B,
        )
        from contextlib import ExitStack

        with tile.TileContext(nc) as tc:
            with ExitStack() as ctx:
                build_kernel(ctx, tc)
        nc.compile()
        _NC = nc
    return _NC


def _host_layout(vertices, faces):
    """Pure indexing: gather per-face-corner vertices, layout [3, 5120] where
    column n = ft*512 + k*128 + p holds corner k of face ft*128+p."""
    faces4 = np.concatenate([faces, faces[:, :1]], axis=1)  # [1280, 4]
    vidx = faces4.reshape(NTILE, 128, 4).transpose(0, 2, 1).reshape(-1)
    out = []
    for b in range(B):
        vg = vertices[b][vidx]  # [5120, 3]
        v4 = np.concatenate(
            [vg.T.astype(np.float32),
             np.full((1, len(vidx)), -1.0, np.float32)], axis=0)
        out.append(np.ascontiguousarray(v4))
    return out


def kernel(vertices, viewpoints, faces, img_size):
    vertices = np.asarray(vertices, dtype=np.float32)
    viewpoints = np.asarray(viewpoints, dtype=np.float32)
    faces = np.asarray(faces, dtype=np.int32)
    assert int(img_size) == IMG and vertices.shape == (B, V, 3)

    nc = _get_program()
    vgts = _host_layout(vertices, faces)
    in_maps = [
        {"vgt": vgts[b], "eye": np.ascontiguousarray(viewpoints[b])}
        for b in range(B)
    ]
    res = run_bass_kernel_spmd(nc, in_maps, core_ids=list(range(B)))
    sil = np.stack([res.results[b]["sil"] for b in range(B)])  # [8, 4096]
    return sil.reshape(B, 1, IMG, IMG).astype(np.float32)


if __name__ == "__main__":
    rng = np.random.default_rng(0)
    verts = rng.standard_normal((B, V, 3), dtype=np.float32) * 0.5
    vps = rng.standard_normal((B, 3), dtype=np.float32)
    fcs = rng.integers(0, V, (NF, 3), dtype=np.int32)
    out = kernel(verts, vps, fcs, IMG)
    print(out.shape, out.sum())
